# revision 1
# baseline (speedup 1.0000x reference)
"""Trainium2 Bass kernel: BigramHashEmbedding (hash -> embed gather -> proj -> scale).

Computation (per batch row, one NeuronCore per row, 8 rows total):
    h[0]  = 10239
    h[j]  = (36313*t[j] ^ 27191*t[j-1]) % 10239          (int32, j >= 1)
    e     = embed_weight[h]                               [S, 128] gather
    out   = (e @ proj_weight.T) * scale                   [S, 512]

Device strategy per core (S = 8192 tokens):
  * tokens are viewed int32 (lo-words of int64 if needed) and loaded into
    SBUF in [16, 512] layout (partition p holds tokens 512p..512p+511),
    replicated 8x across the 128 partitions via one broadcast DMA (the
    dma_gather index tile must be wrapped in 16 partitions and replicated).
  * the bigram hash runs on DVE/ACT with fp32-exact arithmetic: products are
    split (36313 = 141*256 + 217, 27191 = 106*256 + 55) so every arithmetic
    op stays below 2^24 (the vector ALU is fp32 internally); >=2^24 values
    only pass through bitwise ops, which are bit-exact.  mod-10239 is a limb
    decomposition X = u*2^21 + v*2^8 + w -> y = u*8396 + (v<<8) + w (y < 2^24)
    plus one fp32 reciprocal-multiply quotient; the HW float->int converter
    rounds to nearest, so a single +m fixup suffices (r is always < m).
  * the embed table is converted once to bf16 in DRAM (cast-during-DMA on
    SWDGE); eight dma_gathers (1024 rows each, parallel SWDGE queues) fetch
    rows into [128 slots, 64, 128] bf16 (slot k%128 / block k//128; slot k
    holds token 512*(k%16) + k//16).  bf16 keeps the PE off the fp32 power
    throttle (HAM k=4) and halves gather traffic; output rel-err ~3e-3.
  * per 128-token block: bf16 PE transpose (identity) -> PSUM -> bf16 eT in
    SBUF (DVE copy), then PE matmul eT.T @ projT_bf16 -> PSUM f32 ->
    SBUF (ACT/DVE alternating) -> HWDGE DMA to the strided output rows.
    Emission is software-pipelined (transpose runs LAG blocks ahead of the
    matmul) so the eT copy stays off the PE's in-order critical path.
  * proj [512, 128] is transposed on the PE at setup into projT [128, 512],
    pre-scaled by `scale` (broadcast via a K=1 matmul), then cast to bf16.

SWDGE semaphore lanes are round-robin (8) and lock to one queue each, so
every SWDGE DMA uses queue = emission_index % N_QUEUES to keep lane->queue
stable across the wrap.
"""

from contextlib import ExitStack

import numpy as np

import concourse.bacc as bacc
import concourse.bass as bass
import concourse.mybir as mybir
import concourse.tile as tile
from concourse.bass_utils import run_bass_kernel_spmd
from concourse.masks import make_identity

AL = mybir.AluOpType
F32 = mybir.dt.float32
BF16 = mybir.dt.bfloat16
I32 = mybir.dt.int32
I16 = mybir.dt.int16

B = 8           # batch rows == cores
S = 8192        # tokens per core
V = 10240       # hash table rows
D = 128         # embed dim
M = 512         # model dim
P = 128
MOD = 10239     # hash modulus (HASH_SIZE - 1)
SPT = S // 16   # tokens per index-partition = 512
NG = 8          # sub-gathers
TPG = S // NG   # tokens per gather = 1024
CPG = SPT // NG  # idx columns per gather = 64
NB = S // P     # 128-token blocks = 64
BPG = NB // NG  # blocks per gather = 8
HASH_CHUNKS = (64, 64, 128, 256)   # progressive: short first chain, wide later
assert sum(HASH_CHUNKS) == SPT

# 36313 = 141*256 + 217 ; 27191 = 106*256 + 55
A_HI, A_LO = 141, 217
B_HI, B_LO = 106, 55
C21 = 8396      # 2^21 mod 10239
INV_M = 1.0 / MOD

USE_ACT_MUL = True   # run the big hash multiplies on the Scalar (ACT) engine
N_QUEUES = 4         # SWDGE queues
SIM_COMPAT = False   # add the >=MOD fixup (only needed under CoreSim's trunc convert)
LAG = 6              # transpose runs LAG blocks ahead of the matmul


def _mul(nc, out, in_, const):
    if USE_ACT_MUL:
        nc.scalar.mul(out, in_, float(const))
    else:
        nc.vector.tensor_scalar_mul(out, in_, float(const))


def _hash_chunk(nc, tmp, idx, toks_v, tm1, mask, offs, cs, n):
    """Emit ops computing idx[:, cs:cs+n] (int16 hash values).

    toks_v: [128, SPT, W] int32 view of the token tile (lo word at w=0).
    tm1:    [128, 1] int32, t[512p - 1] per partition (garbage at p%16==0).
    mask:   [128, 1] int32, (p % 16) != 0.
    offs:   [128, 1] int32, 10239 * (p % 16 == 0).
    """
    head = cs == 0  # only the first chunk handles the row-head token

    tcur = toks_v[:, cs:cs + n, 0:1]
    p1 = tmp.tile([P, n], I32, tag=f"p1_{n}")
    p2 = tmp.tile([P, n], I32, tag=f"p2_{n}")
    q1 = tmp.tile([P, n], I32, tag=f"q1_{n}")
    q2 = tmp.tile([P, n], I32, tag=f"q2_{n}")
    _mul(nc, p1[:], tcur, A_LO)
    _mul(nc, p2[:], tcur, A_HI)
    if head:
        tprev = toks_v[:, 0:n - 1, 0:1]
        _mul(nc, q1[:, 1:n], tprev, B_LO)
        _mul(nc, q2[:, 1:n], tprev, B_HI)
        _mul(nc, q1[:, 0:1], tm1[:], B_LO)
        _mul(nc, q2[:, 0:1], tm1[:], B_HI)
    else:
        tprev = toks_v[:, cs - 1:cs + n - 1, 0:1]
        _mul(nc, q1[:], tprev, B_LO)
        _mul(nc, q2[:], tprev, B_HI)

    # A>>8 = p2 + (p1>>8);  B>>8 = q2 + (q1>>8)   (both < 2^23, exact)
    # (the compiler rejects bitwise op0 fused with arith op1, so shift and
    # add are separate instructions)
    ah = tmp.tile([P, n], I32, tag=f"ah_{n}")
    bh = tmp.tile([P, n], I32, tag=f"bh_{n}")
    t1 = tmp.tile([P, n], I32, tag=f"t1_{n}")
    nc.vector.tensor_single_scalar(t1[:], p1[:], 8, op=AL.logical_shift_right)
    nc.vector.tensor_add(ah[:], t1[:], p2[:])
    nc.vector.tensor_single_scalar(t1[:], q1[:], 8, op=AL.logical_shift_right)
    nc.vector.tensor_add(bh[:], t1[:], q2[:])
    # X>>8 and X low byte (in low 8 bits of xl)
    xh = tmp.tile([P, n], I32, tag=f"xh_{n}")
    xl = tmp.tile([P, n], I32, tag=f"xl_{n}")
    nc.vector.tensor_tensor(xh[:], ah[:], bh[:], op=AL.bitwise_xor)
    nc.vector.tensor_tensor(xl[:], p1[:], q1[:], op=AL.bitwise_xor)

    # y = (xh>>13)*8396 + ((xh & 8191) << 8) + (xl & 255)   ( < 2^24 )
    w1 = tmp.tile([P, n], I32, tag=f"w1_{n}")
    w2 = tmp.tile([P, n], I32, tag=f"w2_{n}")
    nc.vector.tensor_single_scalar(w1[:], xh[:], 13, op=AL.logical_shift_right)
    nc.vector.tensor_scalar_mul(w1[:], w1[:], float(C21))
    nc.vector.tensor_scalar(w2[:], xh[:], 8191, 8,
                            op0=AL.bitwise_and, op1=AL.logical_shift_left)
    w3 = tmp.tile([P, n], I32, tag=f"w3_{n}")
    nc.vector.tensor_add(w3[:], w1[:], w2[:])
    y = tmp.tile([P, n], I32, tag=f"y_{n}")
    nc.vector.tensor_single_scalar(y[:], xl[:], 255, op=AL.bitwise_and)
    nc.vector.tensor_add(y[:], y[:], w3[:])

    # r = y - rne(y/m)*m  (HW converter is round-to-nearest => r < m always)
    qt = tmp.tile([P, n], I32, tag=f"qt_{n}")
    _mul(nc, qt[:], y[:], INV_M)
    r = tmp.tile([P, n], I32, tag=f"r_{n}")
    nc.vector.scalar_tensor_tensor(r[:], qt[:], -float(MOD), y[:],
                                   op0=AL.mult, op1=AL.add)
    if SIM_COMPAT:
        f1 = tmp.tile([P, n], I32, tag=f"f1_{n}")
        nc.vector.tensor_single_scalar(f1[:], r[:], float(MOD), op=AL.is_ge)
        nc.vector.scalar_tensor_tensor(r[:], f1[:], -float(MOD), r[:],
                                       op0=AL.mult, op1=AL.add)
    f2 = tmp.tile([P, n], I32, tag=f"f2_{n}")
    nc.vector.tensor_single_scalar(f2[:], r[:], 0.0, op=AL.is_lt)
    nc.vector.scalar_tensor_tensor(r[:], f2[:], float(MOD), r[:],
                                   op0=AL.mult, op1=AL.add)

    if head:
        # token 0 (partition p%16==0, col 0): h = MOD
        nc.vector.tensor_mul(r[:, 0:1], r[:, 0:1], mask[:])
        nc.vector.tensor_add(r[:, 0:1], r[:, 0:1], offs[:])

    nc.vector.tensor_copy(idx[:, cs:cs + n], r[:])


def body(ctx: ExitStack, tc: tile.TileContext, out_ap, tok_ap, table_ap,
         proj_ap, scale_ap, W: int):
    """Emit the per-core kernel. tok_ap is int32 [S*W] (W=2 -> int64 lo/hi)."""
    nc = tc.nc

    const = ctx.enter_context(tc.tile_pool(name="const", bufs=1))
    tmp = ctx.enter_context(tc.tile_pool(name="tmp", bufs=2))
    gpool = ctx.enter_context(tc.tile_pool(name="gpool", bufs=1))
    et_pool = ctx.enter_context(tc.tile_pool(name="et", bufs=6))
    o_pool = ctx.enter_context(tc.tile_pool(name="osb", bufs=3))
    dram = ctx.enter_context(tc.tile_pool(name="dram", bufs=1, space="DRAM"))

    # one-time bf16 table conversion in DRAM (cast-during-DMA on SWDGE) --
    # emitted first: every gather depends on it.
    # SWDGE queue discipline: queue = emission_index % N_QUEUES (module doc).
    table_bf = dram.tile([V, D], BF16)
    nc.gpsimd.dma_start(table_bf[:], table_ap)
    swdge_i = 1

    # ---- tokens (they gate the hash -> gather critical path) ----
    FW = SPT * W
    tokv = tok_ap.rearrange("(p f) -> p f", p=16)
    toks = const.tile([P, FW], I32)
    tm1 = const.tile([P, W], I32)
    nc.gpsimd.memset(tm1[:], 0)
    nc.sync.dma_start(toks[:], tokv[None].broadcast_to([8, 16, FW]))
    for r in range(8):
        # t[512q - 1] for q>=1: last element of the previous partition
        nc.sync.dma_start(tm1[16 * r + 1:16 * (r + 1), :],
                          tokv[0:15, FW - W:FW])
    toks_v = toks.rearrange("p (s w) -> p s w", w=W)

    # partition masks for the token-0 override
    pi = const.tile([P, 1], I32)
    nc.gpsimd.iota(pi[:], pattern=[[0, 1]], base=0, channel_multiplier=1)
    mask = const.tile([P, 1], I32)
    nc.vector.tensor_single_scalar(mask[:], pi[:], 15, op=AL.bitwise_and)
    nc.vector.tensor_single_scalar(mask[:], mask[:], 0.0, op=AL.not_equal)
    offs = const.tile([P, 1], I32)
    nc.vector.tensor_scalar(offs[:], mask[:], -float(MOD), float(MOD),
                            op0=AL.mult, op1=AL.add)

    idx = const.tile([P, SPT], I16)
    g_sb = gpool.tile([P, NB, P], BF16)

    # hash + gathers (each chunk covers whole gathers; gather = CPG columns)
    cs = 0
    for n in HASH_CHUNKS:
        _hash_chunk(nc, tmp, idx, toks_v, tm1[:, 0:1], mask, offs, cs, n)
        for g in range(cs // CPG, (cs + n) // CPG):
            nc.gpsimd.dma_gather(
                g_sb[:, BPG * g:BPG * (g + 1), :],
                table_bf[:],
                idx[:, CPG * g:CPG * (g + 1)],
                num_idxs=TPG,
                num_idxs_reg=TPG,
                elem_size=D,
                single_packet=False,
                queue_num=swdge_i % N_QUEUES,
            )
            swdge_i += 1
        cs += n

    # ---- setup: identity, projT (transposed, pre-scaled, bf16) ----
    ps_setup = tc.alloc_tile_pool(name="ps_setup", bufs=1, space="PSUM")
    ident_f = const.tile([P, P], F32)
    make_identity(nc, ident_f[:])
    ident = const.tile([P, P], BF16)
    nc.vector.tensor_copy(ident[:], ident_f[:])

    # scale broadcast [1,1] -> [128,1] via K=1 matmul with a ones row
    sc_in = const.tile([1, 1], F32)
    nc.sync.dma_start(sc_in[:], scale_ap)
    ones = const.tile([1, P], F32)
    nc.gpsimd.memset(ones[:], 1.0)
    ps_sc = ps_setup.tile([P, 1], F32, space="PSUM", tag="ps_sc")
    nc.tensor.matmul(ps_sc[:], lhsT=ones[:], rhs=sc_in[:], start=True, stop=True)
    sc_b = const.tile([P, 1], F32)
    nc.vector.tensor_copy(sc_b[:], ps_sc[:])

    projT = const.tile([P, M], F32)
    for c in range(M // P):
        pch = tmp.tile([P, P], F32, tag="pch")
        nc.sync.dma_start(pch[:], proj_ap[c * P:(c + 1) * P, :])
        ps_t = ps_setup.tile([P, P], F32, space="PSUM", tag="ps_t")
        nc.tensor.transpose(ps_t[:], pch[:], ident_f[:])
        nc.vector.tensor_copy(projT[:, c * P:(c + 1) * P], ps_t[:])
    nc.vector.tensor_scalar_mul(projT[:], projT[:], sc_b[:, 0:1])
    projT_b = const.tile([P, M], BF16)
    nc.vector.tensor_copy(projT_b[:], projT[:])
    ps_setup.release()

    ps_small = ctx.enter_context(tc.tile_pool(name="ps_small", bufs=4, space="PSUM"))
    ps_big = ctx.enter_context(tc.tile_pool(name="ps_big", bufs=4, space="PSUM"))

    # Output-partition remap: the eT cast permutes the free (slot) dim so the
    # matmul's out partition p = 8q + r (token 512q + 8s + r).  The DRAM AP
    # then iterates q-outer / r-inner, which makes each group of 8 (and with
    # 4-block grouping, 32) consecutive descriptors cover a contiguous 16KB
    # (64KB) DRAM run -- strided-descriptor HBM writes measured 176 GB/s vs
    # 301 GB/s for contiguous runs.
    out_q = out_ap.rearrange("(q s r) m -> q r s m", q=16, s=NB, r=8)
    GRP = 1
    # ps_et col for new slot snew=8q+r is slot = q + 16r (q=snew//8, r=snew%8)
    ets = {}
    o4s = {}

    def emit_trans(b):
        ps_et = ps_small.tile([P, P], BF16, space="PSUM",
                              tag="ps_et", name=f"ps_et{b}")
        nc.tensor.transpose(ps_et[:], g_sb[:, b, :], ident[:])
        et = et_pool.tile([P, P], BF16, tag="et", name=f"et{b}")
        src = ps_et.rearrange("d (r q) -> d q r", q=16)  # col q+16r at [q, r]
        nc.vector.tensor_copy(et[:], src)
        ets[b] = et

    def emit_mm(b):
        et = ets.pop(b)
        gi, gb = divmod(b, GRP)
        if gb == 0:
            o4s[gi] = o_pool.tile([P, GRP, M], F32, tag="o_sb", name=f"o4_{gi}")
        o4 = o4s[gi]
        ps_o = ps_big.tile([P, M], F32, space="PSUM", tag="ps_o",
                           name=f"ps_o{b}")
        nc.tensor.matmul(ps_o[:], lhsT=et[:], rhs=projT_b[:],
                         start=True, stop=True)
        nc.scalar.copy(o4[:, gb, :], ps_o[:])
        if gb == GRP - 1:
            nc.sync.dma_start(out_q[:, :, GRP * gi:GRP * (gi + 1), :], o4[:])
            del o4s[gi]

    for b in range(NB):
        emit_trans(b)
        if b >= LAG:
            emit_mm(b - LAG)
    for b in range(NB - LAG, NB):
        emit_mm(b)


_CACHE: dict = {}


def _build(W: int):
    if W in _CACHE:
        return _CACHE[W]
    nc = bacc.Bacc("TRN2", target_bir_lowering=False, debug=False,
                   num_swdge_queues=N_QUEUES, dynamic_dma_scratch_size=65536)
    tok = nc.dram_tensor("token_ids", [S * W], I32, kind="ExternalInput").ap()
    table = nc.dram_tensor("embed_weight", [V, D], F32, kind="ExternalInput").ap()
    proj = nc.dram_tensor("proj_weight", [M, D], F32, kind="ExternalInput").ap()
    scale = nc.dram_tensor("scale", [1, 1], F32, kind="ExternalInput").ap()
    out = nc.dram_tensor("out", [S, M], F32, kind="ExternalOutput").ap()
    with tile.TileContext(nc) as tc:
        with ExitStack() as ctx:
            body(ctx, tc, out, tok, table, proj, scale, W)
    nc.compile()
    _CACHE[W] = nc
    return nc


def kernel(token_ids: np.ndarray, embed_weight: np.ndarray,
           proj_weight: np.ndarray, scale: np.ndarray) -> np.ndarray:
    token_ids = np.ascontiguousarray(token_ids)
    assert token_ids.shape == (B, S), token_ids.shape
    W = 2 if token_ids.dtype.itemsize == 8 else 1
    tok32 = token_ids.view(np.int32).reshape(B, S * W)
    table = np.ascontiguousarray(embed_weight, dtype=np.float32)
    proj = np.ascontiguousarray(proj_weight, dtype=np.float32)
    sc = np.asarray(scale, dtype=np.float32).reshape(1, 1)

    nc = _build(W)
    in_maps = [
        {
            "token_ids": np.ascontiguousarray(tok32[i]),
            "embed_weight": table,
            "proj_weight": proj,
            "scale": sc,
        }
        for i in range(B)
    ]
    res = run_bass_kernel_spmd(nc, in_maps, core_ids=list(range(B)))
    return np.stack([r["out"] for r in res.results], axis=0)



# revision 10
# speedup vs baseline: 1.0793x; 1.0793x over previous
"""Trainium2 Bass kernel: BigramHashEmbedding (hash -> embed gather -> proj -> scale).

Computation (per batch row, one NeuronCore per row, 8 rows total):
    h[0]  = 10239
    h[j]  = (36313*t[j] ^ 27191*t[j-1]) % 10239          (int32, j >= 1)
    e     = embed_weight[h]                               [S, 128] gather
    out   = (e @ proj_weight.T) * scale                   [S, 512]

Device strategy per core (S = 8192 tokens):
  * tokens are loaded in a 16-wrap layout: t16[p, s] = t[16s + p] (p in
    0..15), because dma_gather unwraps its index tile column-major over 16
    partitions (slot k <- idx[k%16, k//16]).  With this layout slot k maps
    to token k exactly, so gathered data, matmul outputs and DRAM writes
    are all in plain token order (fully contiguous output DMA).
    The "previous token" tile is the same data shifted by one: rows 1..15
    come from rows 0..14, row 0 from row 15 shifted one column.  Both
    tiles are broadcast x8 across the 128 partitions (the gather needs the
    idx rows replicated, and the hash then runs on all 128 DVE lanes).
  * the bigram hash runs on DVE/ACT with fp32-exact arithmetic: products
    are split (36313 = 141*256 + 217, 27191 = 106*256 + 55) so every
    arithmetic op stays below 2^24; >=2^24 values only pass through
    bitwise ops.  mod-10239 is a limb decomposition plus one fp32
    reciprocal-multiply quotient (RNE convert => a single +m fixup).
  * the embed table is converted once to bf16 in DRAM (cast-during-DMA on
    SWDGE).  Eight TRANSPOSED dma_gathers (1024 rows each) fetch rows
    directly into [128 dims, 1024 tokens] bf16 tiles -- the DMA itself
    does the 16-bit-granularity transpose, so the PE never transposes
    per-token data at all.
  * per 128-token block: one PE matmul eT_chunk.T @ projT_bf16 -> PSUM
    f32 -> engine copy (rotating ACT/Pool/DVE) casting to bf16 in SBUF ->
    HWDGE DMA (rotating SP/ACT/DVE) to contiguous out rows.  The output
    tensor is bf16; the host upcasts to f32 (tolerance is ~2e-2, bf16
    rounding adds ~4e-3).
  * proj [512, 128] is transposed on the PE at setup into projT [128,
    512], pre-scaled by `scale` (broadcast via a K=1 matmul), cast bf16.

SWDGE semaphore lanes are round-robin (8) and lock to one queue each, so
every SWDGE DMA uses queue = emission_index % N_QUEUES to keep lane->queue
stable across the wrap.
"""

from contextlib import ExitStack

import numpy as np

import concourse.bacc as bacc
import concourse.bass as bass
import concourse.mybir as mybir
import concourse.tile as tile
from concourse.bass_utils import run_bass_kernel_spmd
from concourse.masks import make_identity

AL = mybir.AluOpType
F32 = mybir.dt.float32
BF16 = mybir.dt.bfloat16
I32 = mybir.dt.int32
I16 = mybir.dt.int16

B = 8           # batch rows == cores
S = 8192        # tokens per core
V = 10240       # hash table rows
D = 128         # embed dim
M = 512         # model dim
P = 128
MOD = 10239     # hash modulus (HASH_SIZE - 1)
SPT = S // 16   # 16-wrap columns = 512
NG = 8          # gathers
IPG = S // NG   # idxs per gather = 1024
CPG = IPG // 16  # idx columns per gather = 64
NB = S // P     # 128-token blocks = 64
BPG = IPG // P  # matmul blocks per gather = 8
HASH_CHUNKS = (64, 64, 128, 256)   # progressive: short first chain, wide later
assert sum(HASH_CHUNKS) == SPT

# 36313 = 141*256 + 217 ; 27191 = 106*256 + 55
A_HI, A_LO = 141, 217
B_HI, B_LO = 106, 55
C21 = 8396      # 2^21 mod 10239
INV_M = 1.0 / MOD

USE_ACT_MUL = True   # run the big hash multiplies on the Scalar (ACT) engine
N_QUEUES = 4         # SWDGE queues (ucode MAX_SWDGE_QUEUES=4)
SIM_COMPAT = False   # add the >=MOD fixup (only needed under CoreSim's trunc convert)


def _mul(nc, out, in_, const):
    if USE_ACT_MUL:
        nc.scalar.mul(out, in_, float(const))
    else:
        nc.vector.tensor_scalar_mul(out, in_, float(const))


def _hash_chunk(nc, tmp, idx, cur, prv, mask, offs, cs, n):
    """Emit ops computing idx[:, cs:cs+n] (int16 hash values).

    cur: [128, SPT] int32, cur[p, s] = t[16s + p%16]   (x8 replicas)
    prv: [128, SPT] int32, prv[p, s] = t[16s + p%16 - 1] (garbage at (0,0))
    mask: [128, 1] int32, (p % 16) != 0.
    offs: [128, 1] int32, 10239 * (p % 16 == 0).
    """
    tcur = cur[:, cs:cs + n]
    tprev = prv[:, cs:cs + n]
    p1 = tmp.tile([P, n], I32, tag=f"p1_{n}")
    p2 = tmp.tile([P, n], I32, tag=f"p2_{n}")
    q1 = tmp.tile([P, n], I32, tag=f"q1_{n}")
    q2 = tmp.tile([P, n], I32, tag=f"q2_{n}")
    _mul(nc, p1[:], tcur, A_LO)
    _mul(nc, p2[:], tcur, A_HI)
    _mul(nc, q1[:], tprev, B_LO)
    _mul(nc, q2[:], tprev, B_HI)

    # A>>8 = p2 + (p1>>8);  B>>8 = q2 + (q1>>8)   (both < 2^23, exact)
    ah = tmp.tile([P, n], I32, tag=f"ah_{n}")
    bh = tmp.tile([P, n], I32, tag=f"bh_{n}")
    t1 = tmp.tile([P, n], I32, tag=f"t1_{n}")
    nc.vector.tensor_single_scalar(t1[:], p1[:], 8, op=AL.logical_shift_right)
    nc.vector.tensor_add(ah[:], t1[:], p2[:])
    nc.vector.tensor_single_scalar(t1[:], q1[:], 8, op=AL.logical_shift_right)
    nc.vector.tensor_add(bh[:], t1[:], q2[:])
    # X>>8 and X low byte (in low 8 bits of xl)
    xh = tmp.tile([P, n], I32, tag=f"xh_{n}")
    xl = tmp.tile([P, n], I32, tag=f"xl_{n}")
    nc.vector.tensor_tensor(xh[:], ah[:], bh[:], op=AL.bitwise_xor)
    nc.vector.tensor_tensor(xl[:], p1[:], q1[:], op=AL.bitwise_xor)

    # y = (xh>>13)*8396 + ((xh & 8191) << 8) + (xl & 255)   ( < 2^24 )
    w1 = tmp.tile([P, n], I32, tag=f"w1_{n}")
    w2 = tmp.tile([P, n], I32, tag=f"w2_{n}")
    nc.vector.tensor_single_scalar(w1[:], xh[:], 13, op=AL.logical_shift_right)
    nc.vector.tensor_scalar_mul(w1[:], w1[:], float(C21))
    nc.vector.tensor_scalar(w2[:], xh[:], 8191, 8,
                            op0=AL.bitwise_and, op1=AL.logical_shift_left)
    w3 = tmp.tile([P, n], I32, tag=f"w3_{n}")
    nc.vector.tensor_add(w3[:], w1[:], w2[:])
    y = tmp.tile([P, n], I32, tag=f"y_{n}")
    nc.vector.tensor_single_scalar(y[:], xl[:], 255, op=AL.bitwise_and)
    nc.vector.tensor_add(y[:], y[:], w3[:])

    # r = y - rne(y/m)*m  (HW converter is round-to-nearest => r < m always)
    qt = tmp.tile([P, n], I32, tag=f"qt_{n}")
    _mul(nc, qt[:], y[:], INV_M)
    r = tmp.tile([P, n], I32, tag=f"r_{n}")
    nc.vector.scalar_tensor_tensor(r[:], qt[:], -float(MOD), y[:],
                                   op0=AL.mult, op1=AL.add)
    if SIM_COMPAT:
        f1 = tmp.tile([P, n], I32, tag=f"f1_{n}")
        nc.vector.tensor_single_scalar(f1[:], r[:], float(MOD), op=AL.is_ge)
        nc.vector.scalar_tensor_tensor(r[:], f1[:], -float(MOD), r[:],
                                       op0=AL.mult, op1=AL.add)
    f2 = tmp.tile([P, n], I32, tag=f"f2_{n}")
    nc.vector.tensor_single_scalar(f2[:], r[:], 0.0, op=AL.is_lt)
    nc.vector.scalar_tensor_tensor(r[:], f2[:], float(MOD), r[:],
                                   op0=AL.mult, op1=AL.add)

    if cs == 0:
        # token 0 (partition p%16==0, col 0): h = MOD
        nc.vector.tensor_mul(r[:, 0:1], r[:, 0:1], mask[:])
        nc.vector.tensor_add(r[:, 0:1], r[:, 0:1], offs[:])

    nc.vector.tensor_copy(idx[:, cs:cs + n], r[:])


DEBUG = False    # dump idx/eT tiles to DRAM for stage-by-stage checking


def body(ctx: ExitStack, tc: tile.TileContext, out_ap, tok_ap, table_ap,
         proj_ap, scale_ap, W: int, dbg=None):
    """Emit the per-core kernel. tok_ap is int32 [S*W] (W=2 -> int64 lo/hi)."""
    nc = tc.nc

    const = ctx.enter_context(tc.tile_pool(name="const", bufs=1))
    tmp = ctx.enter_context(tc.tile_pool(name="tmp", bufs=2))
    gpool = ctx.enter_context(tc.tile_pool(name="gpool", bufs=1))
    o_pool = ctx.enter_context(tc.tile_pool(name="osb", bufs=6))
    dram = ctx.enter_context(tc.tile_pool(name="dram", bufs=1, space="DRAM"))

    # one-time bf16 table conversion in DRAM (cast-during-DMA on SWDGE) --
    # emitted first: every gather depends on it.
    # SWDGE queue discipline: queue = emission_index % N_QUEUES (module doc).
    table_bf = dram.tile([V, D], BF16)
    nc.gpsimd.dma_start(table_bf[:], table_ap)
    swdge_i = 1

    # ---- tokens: 16-wrap layout [16, SPT], lo words only ----
    if W == 1:
        tok_src = tok_ap.rearrange("(s p) -> p s", p=16)
    else:
        tok_src = tok_ap.rearrange("(s p w) -> p s w", p=16, w=W)[:, :, 0:1]
    t16 = const.tile([16, SPT], I32)
    t16_dst = t16[:] if W == 1 else t16[:].rearrange("p (s w) -> p s w", w=1)
    nc.sync.dma_start(t16_dst, tok_src)
    p16 = const.tile([16, SPT], I32)
    nc.gpsimd.memset(p16[0:1, 0:1], 0)
    nc.sync.dma_start(p16[1:16, :], t16[0:15, :])
    nc.sync.dma_start(p16[0:1, 1:SPT], t16[15:16, 0:SPT - 1])
    cur = const.tile([P, SPT], I32)
    prv = const.tile([P, SPT], I32)
    for r in range(8):
        (nc.sync if r % 2 else nc.scalar).dma_start(
            cur[16 * r:16 * (r + 1), :], t16[:])
        (nc.scalar if r % 2 else nc.sync).dma_start(
            prv[16 * r:16 * (r + 1), :], p16[:])

    # partition masks for the token-0 override
    pi = const.tile([P, 1], I32)
    nc.gpsimd.iota(pi[:], pattern=[[0, 1]], base=0, channel_multiplier=1)
    mask = const.tile([P, 1], I32)
    nc.vector.tensor_single_scalar(mask[:], pi[:], 15, op=AL.bitwise_and)
    nc.vector.tensor_single_scalar(mask[:], mask[:], 0.0, op=AL.not_equal)
    offs = const.tile([P, 1], I32)
    nc.vector.tensor_scalar(offs[:], mask[:], -float(MOD), float(MOD),
                            op0=AL.mult, op1=AL.add)

    idx = const.tile([P, SPT], I16)
    # eT[d, g, k] = embed_bf16[h(token 1024g + k), d]
    eT = gpool.tile([P, NG, IPG], BF16)

    # hash + transposed gathers (chunks cover whole gathers; CPG cols each)
    cs = 0
    for n in HASH_CHUNKS:
        _hash_chunk(nc, tmp, idx, cur, prv, mask, offs, cs, n)
        for g in range(cs // CPG, (cs + n) // CPG):
            nc.gpsimd.dma_gather(
                eT[:, g:g + 1, :],
                table_bf[:],
                idx[:, CPG * g:CPG * (g + 1)],
                num_idxs=IPG,
                num_idxs_reg=IPG,
                elem_size=D,
                transpose=True,
                single_packet=False,
                # All transposed gathers share ONE queue: the transpose
                # routes through the shared XBAR via paired TX/RX
                # descriptor streams, and two in-flight transposed gathers
                # on different queues interleave in the xbar FIFO and swap
                # rows (measured).  Same-queue rings execute in order.
                queue_num=1,
            )
            swdge_i += 1
        cs += n

    if dbg is not None:
        nc.sync.dma_start(dbg["idx"], idx[:])
        nc.sync.dma_start(dbg["eT"], eT[:])
        nc.sync.dma_start(dbg["cur"], cur[:])
        nc.sync.dma_start(dbg["prv"], prv[:])

    # ---- setup: identity, projT (transposed, pre-scaled, bf16) ----
    ps_setup = tc.alloc_tile_pool(name="ps_setup", bufs=1, space="PSUM")
    ident_f = const.tile([P, P], F32)
    make_identity(nc, ident_f[:])

    # scale broadcast [1,1] -> [128,1] via K=1 matmul with a ones row
    sc_in = const.tile([1, 1], F32)
    nc.sync.dma_start(sc_in[:], scale_ap)
    ones = const.tile([1, P], F32)
    nc.gpsimd.memset(ones[:], 1.0)
    ps_sc = ps_setup.tile([P, 1], F32, space="PSUM", tag="ps_sc")
    nc.tensor.matmul(ps_sc[:], lhsT=ones[:], rhs=sc_in[:], start=True, stop=True)
    sc_b = const.tile([P, 1], F32)
    nc.vector.tensor_copy(sc_b[:], ps_sc[:])

    projT = const.tile([P, M], F32)
    for c in range(M // P):
        pch = tmp.tile([P, P], F32, tag="pch")
        nc.sync.dma_start(pch[:], proj_ap[c * P:(c + 1) * P, :])
        ps_t = ps_setup.tile([P, P], F32, space="PSUM", tag="ps_t")
        nc.tensor.transpose(ps_t[:], pch[:], ident_f[:])
        nc.vector.tensor_copy(projT[:, c * P:(c + 1) * P], ps_t[:])
    nc.vector.tensor_scalar_mul(projT[:], projT[:], sc_b[:, 0:1])
    projT_b = const.tile([P, M], BF16)
    nc.vector.tensor_copy(projT_b[:], projT[:])
    ps_setup.release()

    ps_big = ctx.enter_context(tc.tile_pool(name="ps_big", bufs=6, space="PSUM"))

    # main loop: matmul -> PSUM -> bf16 copy (ACT/DVE alternating; Pool has
    # no PSUM access) -> DMA out (SP/ACT alternating HWDGE dispatch, paired
    # so ACT never does copy+dispatch for the same block)
    copy_engs = (nc.scalar, nc.vector)
    dma_engs = (nc.sync, nc.scalar)
    for b in range(NB):
        g, j = divmod(b, BPG)
        ps_o = ps_big.tile([P, M], F32, space="PSUM", tag="ps_o",
                           name=f"ps_o{b}")
        nc.tensor.matmul(ps_o[:], lhsT=eT[:, g, P * j:P * (j + 1)],
                         rhs=projT_b[:], start=True, stop=True)
        ob = o_pool.tile([P, M], BF16, tag="ob", name=f"ob{b}")
        ce = copy_engs[b % 2]
        if ce is nc.scalar:
            ce.copy(ob[:], ps_o[:])
        else:
            ce.tensor_copy(ob[:], ps_o[:])
        dma_engs[b % 2].dma_start(out_ap[P * b:P * (b + 1), :], ob[:])


_CACHE: dict = {}


def _build(W: int):
    if W in _CACHE:
        return _CACHE[W]
    nc = bacc.Bacc("TRN2", target_bir_lowering=False, debug=False,
                   num_swdge_queues=N_QUEUES, dynamic_dma_scratch_size=131072)
    tok = nc.dram_tensor("token_ids", [S * W], I32, kind="ExternalInput").ap()
    table = nc.dram_tensor("embed_weight", [V, D], F32, kind="ExternalInput").ap()
    proj = nc.dram_tensor("proj_weight", [M, D], F32, kind="ExternalInput").ap()
    scale = nc.dram_tensor("scale", [1, 1], F32, kind="ExternalInput").ap()
    out = nc.dram_tensor("out", [S, M], BF16, kind="ExternalOutput").ap()
    dbg = None
    if DEBUG:
        dbg = {
            "idx": nc.dram_tensor("idx_dbg", [P, SPT], I16,
                                  kind="ExternalOutput").ap(),
            "eT": nc.dram_tensor("eT_dbg", [P, NG, IPG], BF16,
                                 kind="ExternalOutput").ap(),
            "cur": nc.dram_tensor("cur_dbg", [P, SPT], I32,
                                  kind="ExternalOutput").ap(),
            "prv": nc.dram_tensor("prv_dbg", [P, SPT], I32,
                                  kind="ExternalOutput").ap(),
        }
    with tile.TileContext(nc) as tc:
        with ExitStack() as ctx:
            body(ctx, tc, out, tok, table, proj, scale, W, dbg=dbg)
    nc.compile()
    _CACHE[W] = nc
    return nc


def kernel(token_ids: np.ndarray, embed_weight: np.ndarray,
           proj_weight: np.ndarray, scale: np.ndarray) -> np.ndarray:
    token_ids = np.ascontiguousarray(token_ids)
    assert token_ids.shape == (B, S), token_ids.shape
    W = 2 if token_ids.dtype.itemsize == 8 else 1
    tok32 = token_ids.view(np.int32).reshape(B, S * W)
    table = np.ascontiguousarray(embed_weight, dtype=np.float32)
    proj = np.ascontiguousarray(proj_weight, dtype=np.float32)
    sc = np.asarray(scale, dtype=np.float32).reshape(1, 1)

    nc = _build(W)
    in_maps = [
        {
            "token_ids": np.ascontiguousarray(tok32[i]),
            "embed_weight": table,
            "proj_weight": proj,
            "scale": sc,
        }
        for i in range(B)
    ]
    res = run_bass_kernel_spmd(nc, in_maps, core_ids=list(range(B)))
    return np.stack([np.asarray(r["out"]).astype(np.float32)
                     for r in res.results], axis=0)


# revision 11
# speedup vs baseline: 1.2552x; 1.1630x over previous
"""Trainium2 Bass kernel: BigramHashEmbedding (hash -> embed gather -> proj -> scale).

Computation (per batch row, one NeuronCore per row, 8 rows total):
    h[0]  = 10239
    h[j]  = (36313*t[j] ^ 27191*t[j-1]) % 10239          (int32, j >= 1)
    e     = embed_weight[h]                               [S, 128] gather
    out   = (e @ proj_weight.T) * scale                   [S, 512]

Device strategy per core (S = 8192 tokens):
  * dma_gather unwraps its index tile column-major over 16 partitions
    (slot k <- idx[k%16, k//16]), so the host stages tokens in a 16-wrap
    layout (tok16[p, s] = t[16s + p], plus a one-shifted copy for the
    bigram's previous token; both are pure permutations of the int32 index
    tensor, staged as one [2, 16, 512] input).  With this layout gather
    slot k maps to token k exactly: gathered rows land as
    g_sb[p, b, :] = e[token 128b + p], the PE transpose of block b yields
    eT in plain token order, and every output DMA writes 128 contiguous
    rows (fully sequential HBM addresses).
  * the device loads the two wrapped tiles with contiguous 2KB-per-
    partition runs and broadcasts them x8 across the 128 partitions (the
    gather needs its idx rows replicated per GpSimd core pair; the hash
    then runs on all 128 DVE lanes).
  * the bigram hash runs on DVE/ACT with fp32-exact arithmetic: products
    are split (36313 = 141*256 + 217, 27191 = 106*256 + 55) so every
    arithmetic op stays below 2^24 (the vector ALU is fp32 internally);
    >=2^24 values only pass through bitwise ops, which are bit-exact.
    mod-10239 is a limb decomposition X = u*2^21 + v*2^8 + w ->
    y = u*8396 + (v<<8) + w (y < 2^24) plus one fp32 reciprocal-multiply
    quotient; the HW float->int converter rounds to nearest, so a single
    +m fixup suffices.
  * the embed table is converted once to bf16 in DRAM (cast-during-DMA on
    SWDGE, split into 4 queue-parallel chunks to shorten the startup
    serialization).  Eight dma_gathers (1024 rows each) fetch rows into
    [128, 64, 128] bf16.  (The transpose=True gather mode would skip the
    PE transposes below, but it routes through the shared XBAR: its
    descriptor generation costs ~8.5 ns/row and concurrent transposed
    gathers on different queues corrupt each other, so serialized it is
    ~70 us for 8K rows -- measured.  Plain gathers + PE transposes win.)
  * per 128-token block: bf16 PE transpose (identity) -> PSUM -> bf16 eT
    in SBUF (DVE/ACT alternating copy), then PE matmul eT.T @ projT_bf16
    -> PSUM f32 -> bf16 copy into a 2-block SBUF group (DVE/ACT
    alternating) -> one HWDGE DMA per 2 blocks (256KB contiguous).  The
    output tensor is bf16; the host upcasts to f32 (tolerance ~2e-2, bf16
    out adds ~2e-3).  Transposes run LAG blocks ahead of the matmuls so
    the eT copy stays off the PE's in-order critical path.
  * proj [512, 128] is transposed on the PE at setup into projT [128,
    512], pre-scaled by `scale` (broadcast via a K=1 matmul), cast bf16.

SWDGE semaphore lanes are round-robin (8) and lock to one queue each, so
every SWDGE DMA uses queue = emission_index % N_QUEUES to keep lane->queue
stable across the wrap (12 SWDGE DMAs: 4 conversion chunks + 8 gathers).
"""

from contextlib import ExitStack

import numpy as np

import concourse.bacc as bacc
import concourse.bass as bass
import concourse.mybir as mybir
import concourse.tile as tile
from concourse.bass_utils import run_bass_kernel_spmd
from concourse.masks import make_identity

AL = mybir.AluOpType
F32 = mybir.dt.float32
BF16 = mybir.dt.bfloat16
I32 = mybir.dt.int32
I16 = mybir.dt.int16

B = 8           # batch rows == cores
S = 8192        # tokens per core
V = 10240       # hash table rows
D = 128         # embed dim
M = 512         # model dim
P = 128
MOD = 10239     # hash modulus (HASH_SIZE - 1)
SPT = S // 16   # 16-wrap columns = 512
NG = 8          # gathers
IPG = S // NG   # idxs per gather = 1024
CPG = IPG // 16  # idx columns per gather = 64
NB = S // P     # 128-token blocks = 64
BPG = IPG // P  # matmul blocks per gather = 8
NCONV = 4       # table-conversion chunks
HASH_CHUNKS = (64, 64, 128, 256)   # progressive: short first chain, wide later
assert sum(HASH_CHUNKS) == SPT

# 36313 = 141*256 + 217 ; 27191 = 106*256 + 55
A_HI, A_LO = 141, 217
B_HI, B_LO = 106, 55
C21 = 8396      # 2^21 mod 10239
INV_M = 1.0 / MOD

USE_ACT_MUL = True   # run the big hash multiplies on the Scalar (ACT) engine
N_QUEUES = 4         # SWDGE queues (ucode MAX_SWDGE_QUEUES=4)
SIM_COMPAT = False   # add the >=MOD fixup (only needed under CoreSim's trunc convert)
LAG = 4              # transpose runs LAG blocks ahead of the matmul
GRP = 2              # output blocks per DMA (256KB contiguous)


def _mul(nc, out, in_, const):
    if USE_ACT_MUL:
        nc.scalar.mul(out, in_, float(const))
    else:
        nc.vector.tensor_scalar_mul(out, in_, float(const))


def _hash_chunk(nc, tmp, idx, cur, prv, mask, offs, cs, n):
    """Emit ops computing idx[:, cs:cs+n] (int16 hash values).

    cur: [128, SPT] int32, cur[p, s] = t[16s + p%16]   (x8 replicas)
    prv: [128, SPT] int32, prv[p, s] = t[16s + p%16 - 1] (0 at (p%16==0, 0))
    mask: [128, 1] int32, (p % 16) != 0.
    offs: [128, 1] int32, 10239 * (p % 16 == 0).
    """
    tcur = cur[:, cs:cs + n]
    tprev = prv[:, cs:cs + n]
    p1 = tmp.tile([P, n], I32, tag=f"p1_{n}")
    p2 = tmp.tile([P, n], I32, tag=f"p2_{n}")
    q1 = tmp.tile([P, n], I32, tag=f"q1_{n}")
    q2 = tmp.tile([P, n], I32, tag=f"q2_{n}")
    _mul(nc, p1[:], tcur, A_LO)
    _mul(nc, p2[:], tcur, A_HI)
    _mul(nc, q1[:], tprev, B_LO)
    _mul(nc, q2[:], tprev, B_HI)

    # A>>8 = p2 + (p1>>8);  B>>8 = q2 + (q1>>8)   (both < 2^23, exact)
    ah = tmp.tile([P, n], I32, tag=f"ah_{n}")
    bh = tmp.tile([P, n], I32, tag=f"bh_{n}")
    t1 = tmp.tile([P, n], I32, tag=f"t1_{n}")
    nc.vector.tensor_single_scalar(t1[:], p1[:], 8, op=AL.logical_shift_right)
    nc.vector.tensor_add(ah[:], t1[:], p2[:])
    nc.vector.tensor_single_scalar(t1[:], q1[:], 8, op=AL.logical_shift_right)
    nc.vector.tensor_add(bh[:], t1[:], q2[:])
    # X>>8 and X low byte (in low 8 bits of xl)
    xh = tmp.tile([P, n], I32, tag=f"xh_{n}")
    xl = tmp.tile([P, n], I32, tag=f"xl_{n}")
    nc.vector.tensor_tensor(xh[:], ah[:], bh[:], op=AL.bitwise_xor)
    nc.vector.tensor_tensor(xl[:], p1[:], q1[:], op=AL.bitwise_xor)

    # y = (xh>>13)*8396 + ((xh & 8191) << 8) + (xl & 255)   ( < 2^24 )
    w1 = tmp.tile([P, n], I32, tag=f"w1_{n}")
    w2 = tmp.tile([P, n], I32, tag=f"w2_{n}")
    nc.vector.tensor_single_scalar(w1[:], xh[:], 13, op=AL.logical_shift_right)
    nc.vector.tensor_scalar_mul(w1[:], w1[:], float(C21))
    nc.vector.tensor_scalar(w2[:], xh[:], 8191, 8,
                            op0=AL.bitwise_and, op1=AL.logical_shift_left)
    w3 = tmp.tile([P, n], I32, tag=f"w3_{n}")
    nc.vector.tensor_add(w3[:], w1[:], w2[:])
    y = tmp.tile([P, n], I32, tag=f"y_{n}")
    nc.vector.tensor_single_scalar(y[:], xl[:], 255, op=AL.bitwise_and)
    nc.vector.tensor_add(y[:], y[:], w3[:])

    # r = y - rne(y/m)*m  (HW converter is round-to-nearest => r < m always)
    qt = tmp.tile([P, n], I32, tag=f"qt_{n}")
    _mul(nc, qt[:], y[:], INV_M)
    r = tmp.tile([P, n], I32, tag=f"r_{n}")
    nc.vector.scalar_tensor_tensor(r[:], qt[:], -float(MOD), y[:],
                                   op0=AL.mult, op1=AL.add)
    if SIM_COMPAT:
        f1 = tmp.tile([P, n], I32, tag=f"f1_{n}")
        nc.vector.tensor_single_scalar(f1[:], r[:], float(MOD), op=AL.is_ge)
        nc.vector.scalar_tensor_tensor(r[:], f1[:], -float(MOD), r[:],
                                       op0=AL.mult, op1=AL.add)
    f2 = tmp.tile([P, n], I32, tag=f"f2_{n}")
    nc.vector.tensor_single_scalar(f2[:], r[:], 0.0, op=AL.is_lt)
    nc.vector.scalar_tensor_tensor(r[:], f2[:], float(MOD), r[:],
                                   op0=AL.mult, op1=AL.add)

    if cs == 0:
        # token 0 (partition p%16==0, col 0): h = MOD
        nc.vector.tensor_mul(r[:, 0:1], r[:, 0:1], mask[:])
        nc.vector.tensor_add(r[:, 0:1], r[:, 0:1], offs[:])

    nc.vector.tensor_copy(idx[:, cs:cs + n], r[:])


def body(ctx: ExitStack, tc: tile.TileContext, out_ap, tok_ap, table_ap,
         proj_ap, scale_ap, dbg=None):
    """Emit the per-core kernel.  tok_ap is int32 [2, 16, SPT]: the host-
    staged 16-wrap current-token and previous-token tiles."""
    nc = tc.nc

    const = ctx.enter_context(tc.tile_pool(name="const", bufs=1))
    tmp = ctx.enter_context(tc.tile_pool(name="tmp", bufs=2))
    gpool = ctx.enter_context(tc.tile_pool(name="gpool", bufs=1))
    et_pool = ctx.enter_context(tc.tile_pool(name="et", bufs=6))
    o_pool = ctx.enter_context(tc.tile_pool(name="osb", bufs=3))
    dram = ctx.enter_context(tc.tile_pool(name="dram", bufs=1, space="DRAM"))

    # one-time bf16 table conversion in DRAM (cast-during-DMA on SWDGE),
    # 4 queue-parallel chunks; every gather depends on all of them.
    # SWDGE queue discipline: queue = emission_index % N_QUEUES (module doc).
    table_bf = dram.tile([V, D], BF16)
    RPC = V // NCONV
    swdge_i = 0
    for c in range(NCONV):
        nc.gpsimd.dma_start(table_bf[RPC * c:RPC * (c + 1), :],
                            table_ap[RPC * c:RPC * (c + 1), :])
        swdge_i += 1

    # ---- tokens: host-staged 16-wrap tiles, contiguous per-partition ----
    t16 = const.tile([16, SPT], I32)
    p16 = const.tile([16, SPT], I32)
    nc.sync.dma_start(t16[:], tok_ap[0])
    nc.scalar.dma_start(p16[:], tok_ap[1])
    cur = const.tile([P, SPT], I32)
    prv = const.tile([P, SPT], I32)
    for r in range(8):
        (nc.sync if r % 2 else nc.scalar).dma_start(
            cur[16 * r:16 * (r + 1), :], t16[:])
        (nc.scalar if r % 2 else nc.sync).dma_start(
            prv[16 * r:16 * (r + 1), :], p16[:])

    # partition masks for the token-0 override
    pi = const.tile([P, 1], I32)
    nc.gpsimd.iota(pi[:], pattern=[[0, 1]], base=0, channel_multiplier=1)
    mask = const.tile([P, 1], I32)
    nc.vector.tensor_single_scalar(mask[:], pi[:], 15, op=AL.bitwise_and)
    nc.vector.tensor_single_scalar(mask[:], mask[:], 0.0, op=AL.not_equal)
    offs = const.tile([P, 1], I32)
    nc.vector.tensor_scalar(offs[:], mask[:], -float(MOD), float(MOD),
                            op0=AL.mult, op1=AL.add)

    idx = const.tile([P, SPT], I16)
    # g_sb[p, b, :] = embed_bf16[h(token 128b + p), :]
    g_sb = gpool.tile([P, NB, P], BF16)

    # hash + gathers (each chunk covers whole gathers; gather = CPG columns)
    cs = 0
    for n in HASH_CHUNKS:
        _hash_chunk(nc, tmp, idx, cur, prv, mask, offs, cs, n)
        for g in range(cs // CPG, (cs + n) // CPG):
            nc.gpsimd.dma_gather(
                g_sb[:, BPG * g:BPG * (g + 1), :],
                table_bf[:],
                idx[:, CPG * g:CPG * (g + 1)],
                num_idxs=IPG,
                num_idxs_reg=IPG,
                elem_size=D,
                single_packet=False,
                queue_num=swdge_i % N_QUEUES,
            )
            swdge_i += 1
        cs += n

    if dbg is not None:
        nc.sync.dma_start(dbg["idx"], idx[:])
        nc.sync.dma_start(dbg["cur"], cur[:])
        nc.sync.dma_start(dbg["prv"], prv[:])

    # ---- setup: identity, projT (transposed, pre-scaled, bf16) ----
    ps_setup = tc.alloc_tile_pool(name="ps_setup", bufs=1, space="PSUM")
    ident_f = const.tile([P, P], F32)
    make_identity(nc, ident_f[:])
    ident = const.tile([P, P], BF16)
    nc.vector.tensor_copy(ident[:], ident_f[:])

    # scale broadcast [1,1] -> [128,1] via K=1 matmul with a ones row
    sc_in = const.tile([1, 1], F32)
    nc.sync.dma_start(sc_in[:], scale_ap)
    ones = const.tile([1, P], F32)
    nc.gpsimd.memset(ones[:], 1.0)
    ps_sc = ps_setup.tile([P, 1], F32, space="PSUM", tag="ps_sc")
    nc.tensor.matmul(ps_sc[:], lhsT=ones[:], rhs=sc_in[:], start=True, stop=True)
    sc_b = const.tile([P, 1], F32)
    nc.vector.tensor_copy(sc_b[:], ps_sc[:])

    projT = const.tile([P, M], F32)
    for c in range(M // P):
        pch = tmp.tile([P, P], F32, tag="pch")
        nc.sync.dma_start(pch[:], proj_ap[c * P:(c + 1) * P, :])
        ps_t = ps_setup.tile([P, P], F32, space="PSUM", tag="ps_t")
        nc.tensor.transpose(ps_t[:], pch[:], ident_f[:])
        nc.vector.tensor_copy(projT[:, c * P:(c + 1) * P], ps_t[:])
    nc.vector.tensor_scalar_mul(projT[:], projT[:], sc_b[:, 0:1])
    projT_b = const.tile([P, M], BF16)
    nc.vector.tensor_copy(projT_b[:], projT[:])
    ps_setup.release()

    ps_small = ctx.enter_context(tc.tile_pool(name="ps_small", bufs=4, space="PSUM"))
    ps_big = ctx.enter_context(tc.tile_pool(name="ps_big", bufs=4, space="PSUM"))

    # main loop.  Block b holds tokens 128b..128b+127 in order, so matmul
    # outputs are token-contiguous and each GRP-block DMA writes one
    # sequential 256KB DRAM run.  eT and out copies alternate DVE/ACT.
    ets = {}
    o2s = {}

    def emit_trans(b):
        ps_et = ps_small.tile([P, P], BF16, space="PSUM",
                              tag="ps_et", name=f"ps_et{b}")
        nc.tensor.transpose(ps_et[:], g_sb[:, b, :], ident[:])
        et = et_pool.tile([P, P], BF16, tag="et", name=f"et{b}")
        if b % 2:
            nc.scalar.copy(et[:], ps_et[:])
        else:
            nc.vector.tensor_copy(et[:], ps_et[:])
        ets[b] = et

    def emit_mm(b):
        et = ets.pop(b)
        gi, gb = divmod(b, GRP)
        if gb == 0:
            o2s[gi] = o_pool.tile([P, GRP, M], BF16, tag="o_sb",
                                  name=f"o2_{gi}")
        o2 = o2s[gi]
        ps_o = ps_big.tile([P, M], F32, space="PSUM", tag="ps_o",
                           name=f"ps_o{b}")
        nc.tensor.matmul(ps_o[:], lhsT=et[:], rhs=projT_b[:],
                         start=True, stop=True)
        if b % 2:
            nc.vector.tensor_copy(o2[:, gb, :], ps_o[:])
        else:
            nc.scalar.copy(o2[:, gb, :], ps_o[:])
        if gb == GRP - 1:
            # tokens GRP*128*gi .. +GRP*128: [128, GRP, M] -> rows
            # (gb*128 + p) of the GRP*128-token range, partition-major
            dst = out_ap[GRP * P * gi:GRP * P * (gi + 1), :]
            dst = dst.rearrange("(g p) m -> p g m", g=GRP)
            nc.sync.dma_start(dst, o2[:])
            del o2s[gi]

    for b in range(NB):
        emit_trans(b)
        if b >= LAG:
            emit_mm(b - LAG)
    for b in range(NB - LAG, NB):
        emit_mm(b)


_CACHE: dict = {}
DEBUG = False    # dump idx/cur/prv tiles to DRAM for stage checking


def _build(key: int = 0):
    if key in _CACHE:
        return _CACHE[key]
    nc = bacc.Bacc("TRN2", target_bir_lowering=False, debug=False,
                   num_swdge_queues=N_QUEUES, dynamic_dma_scratch_size=131072)
    tok = nc.dram_tensor("token_ids", [2, 16, SPT], I32, kind="ExternalInput").ap()
    table = nc.dram_tensor("embed_weight", [V, D], F32, kind="ExternalInput").ap()
    proj = nc.dram_tensor("proj_weight", [M, D], F32, kind="ExternalInput").ap()
    scale = nc.dram_tensor("scale", [1, 1], F32, kind="ExternalInput").ap()
    out = nc.dram_tensor("out", [S, M], BF16, kind="ExternalOutput").ap()
    dbg = None
    if DEBUG:
        dbg = {
            "idx": nc.dram_tensor("idx_dbg", [P, SPT], I16,
                                  kind="ExternalOutput").ap(),
            "cur": nc.dram_tensor("cur_dbg", [P, SPT], I32,
                                  kind="ExternalOutput").ap(),
            "prv": nc.dram_tensor("prv_dbg", [P, SPT], I32,
                                  kind="ExternalOutput").ap(),
        }
    with tile.TileContext(nc) as tc:
        with ExitStack() as ctx:
            body(ctx, tc, out, tok, table, proj, scale, dbg=dbg)
    nc.compile()
    _CACHE[key] = nc
    return nc


def stage_tokens(row: np.ndarray) -> np.ndarray:
    """[S] int token row -> [2, 16, SPT] int32 16-wrap (cur, prev) tiles."""
    t32 = row.astype(np.int32)          # values < 2^31; lo-word == value
    prev = np.empty_like(t32)
    prev[0] = 0
    prev[1:] = t32[:-1]
    cur_w = np.ascontiguousarray(t32.reshape(SPT, 16).T)
    prv_w = np.ascontiguousarray(prev.reshape(SPT, 16).T)
    return np.ascontiguousarray(np.stack([cur_w, prv_w]))


def kernel(token_ids: np.ndarray, embed_weight: np.ndarray,
           proj_weight: np.ndarray, scale: np.ndarray) -> np.ndarray:
    token_ids = np.ascontiguousarray(token_ids)
    assert token_ids.shape == (B, S), token_ids.shape
    table = np.ascontiguousarray(embed_weight, dtype=np.float32)
    proj = np.ascontiguousarray(proj_weight, dtype=np.float32)
    sc = np.asarray(scale, dtype=np.float32).reshape(1, 1)

    nc = _build()
    in_maps = [
        {
            "token_ids": stage_tokens(token_ids[i]),
            "embed_weight": table,
            "proj_weight": proj,
            "scale": sc,
        }
        for i in range(B)
    ]
    res = run_bass_kernel_spmd(nc, in_maps, core_ids=list(range(B)))
    return np.stack([np.asarray(r["out"]).astype(np.float32)
                     for r in res.results], axis=0)


# revision 16
# speedup vs baseline: 1.3680x; 1.0899x over previous
"""Trainium2 Bass kernel: BigramHashEmbedding (hash -> embed gather -> proj -> scale).

Computation (per batch row, one NeuronCore per row, 8 rows total):
    h[0]  = 10239
    h[j]  = (36313*t[j] ^ 27191*t[j-1]) % 10239          (int32, j >= 1)
    e     = embed_weight[h]                               [S, 128] gather
    out   = (e @ proj_weight.T) * scale                   [S, 512]

Device strategy per core (S = 8192 tokens):
  * dma_gather unwraps its index tile column-major over 16 partitions
    (slot k <- idx[k%16, k//16]), so the host stages tokens in a 16-wrap
    layout (tok16[p, s] = t[16s + p], plus a one-shifted copy for the
    bigram's previous token; both are pure permutations of the int32 index
    tensor, staged as one [2, 16, 512] input).  With this layout gather
    slot k maps to token k exactly: gathered rows land as
    g_sb[p, b, :] = e[token 128b + p], the PE transpose of block b yields
    eT in plain token order, and every output DMA writes 128 contiguous
    rows (fully sequential HBM addresses).
  * the device loads the two wrapped tiles with contiguous 2KB-per-
    partition runs and broadcasts them x8 across the 128 partitions (the
    gather needs its idx rows replicated per GpSimd core pair; the hash
    then runs on all 128 DVE lanes).
  * the bigram hash runs on DVE/ACT with fp32-exact arithmetic: products
    are split (36313 = 141*256 + 217, 27191 = 106*256 + 55) so every
    arithmetic op stays below 2^24 (the vector ALU is fp32 internally);
    >=2^24 values only pass through bitwise ops, which are bit-exact.
    mod-10239 is a limb decomposition X = u*2^21 + v*2^8 + w ->
    y = u*8396 + (v<<8) + w (y < 2^24) plus one fp32 reciprocal-multiply
    quotient; the HW float->int converter rounds to nearest, so a single
    +m fixup suffices.
  * the embed table is converted once to bf16 in DRAM (cast-during-DMA on
    SWDGE, split into 4 queue-parallel chunks to shorten the startup
    serialization).  Eight dma_gathers (1024 rows each) fetch rows into
    [128, 64, 128] bf16.  (The transpose=True gather mode would skip the
    PE transposes below, but it routes through the shared XBAR: its
    descriptor generation costs ~8.5 ns/row and concurrent transposed
    gathers on different queues corrupt each other, so serialized it is
    ~70 us for 8K rows -- measured.  Plain gathers + PE transposes win.)
  * per 128-token block: bf16 PE transpose (identity) -> PSUM -> bf16 eT
    in SBUF (DVE/ACT alternating copy), then PE matmul eT.T @ projT_bf16
    -> PSUM f32 -> bf16 copy into a 2-block SBUF group (DVE/ACT
    alternating) -> one HWDGE DMA per 2 blocks (256KB contiguous).  The
    output tensor is bf16; the host upcasts to f32 (tolerance ~2e-2, bf16
    out adds ~2e-3).  Transposes run LAG blocks ahead of the matmuls so
    the eT copy stays off the PE's in-order critical path.
  * proj [512, 128] is transposed on the PE at setup into projT [128,
    512], pre-scaled by `scale` (broadcast via a K=1 matmul), cast bf16.

SWDGE semaphore lanes are round-robin (8) and lock to one queue each, so
every SWDGE DMA uses queue = emission_index % N_QUEUES to keep lane->queue
stable across the wrap (12 SWDGE DMAs: 4 conversion chunks + 8 gathers).
"""

from contextlib import ExitStack

import numpy as np

import concourse.bacc as bacc
import concourse.bass as bass
import concourse.mybir as mybir
import concourse.tile as tile
from concourse.bass_utils import run_bass_kernel_spmd
from concourse.masks import make_identity

AL = mybir.AluOpType
F32 = mybir.dt.float32
BF16 = mybir.dt.bfloat16
I32 = mybir.dt.int32
I16 = mybir.dt.int16

B = 8           # batch rows == cores
S = 8192        # tokens per core
V = 10240       # hash table rows
D = 128         # embed dim
M = 512         # model dim
P = 128
MOD = 10239     # hash modulus (HASH_SIZE - 1)
SPT = S // 16   # 16-wrap columns = 512
NG = 8          # gathers
IPG = S // NG   # idxs per gather = 1024
CPG = IPG // 16  # idx columns per gather = 64
NB = S // P     # 128-token blocks = 64
BPG = IPG // P  # matmul blocks per gather = 8
NCONV = 4       # table-conversion chunks
HASH_CHUNKS = (64, 64, 128, 256)   # progressive: short first chain, wide later
assert sum(HASH_CHUNKS) == SPT

# 36313 = 141*256 + 217 ; 27191 = 106*256 + 55
A_HI, A_LO = 141, 217
B_HI, B_LO = 106, 55
C21 = 8396      # 2^21 mod 10239
INV_M = 1.0 / MOD

USE_ACT_MUL = True   # run the big hash multiplies on the Scalar (ACT) engine
N_QUEUES = 4         # SWDGE queues (ucode MAX_SWDGE_QUEUES=4)
SIM_COMPAT = False   # add the >=MOD fixup (only needed under CoreSim's trunc convert)
LAG = 4              # transpose runs LAG blocks ahead of the matmul
GRP = 2              # output blocks per DMA (256KB contiguous)


def _mul(nc, out, in_, const):
    if USE_ACT_MUL:
        nc.scalar.mul(out, in_, float(const))
    else:
        nc.vector.tensor_scalar_mul(out, in_, float(const))


def _hash_chunk(nc, tmp, idx, cur, prv, mask, offs, cs, n):
    """Emit ops computing idx[:, cs:cs+n] (int16 hash values).

    cur: [128, SPT] int32, cur[p, s] = t[16s + p%16]   (x8 replicas)
    prv: [128, SPT] int32, prv[p, s] = t[16s + p%16 - 1] (0 at (p%16==0, 0))
    mask: [128, 1] int32, (p % 16) != 0.
    offs: [128, 1] int32, 10239 * (p % 16 == 0).
    """
    tcur = cur[:, cs:cs + n]
    tprev = prv[:, cs:cs + n]
    p1 = tmp.tile([P, n], I32, tag=f"p1_{n}")
    p2 = tmp.tile([P, n], I32, tag=f"p2_{n}")
    q1 = tmp.tile([P, n], I32, tag=f"q1_{n}")
    q2 = tmp.tile([P, n], I32, tag=f"q2_{n}")
    _mul(nc, p1[:], tcur, A_LO)
    _mul(nc, p2[:], tcur, A_HI)
    _mul(nc, q1[:], tprev, B_LO)
    _mul(nc, q2[:], tprev, B_HI)

    # A>>8 = p2 + (p1>>8);  B>>8 = q2 + (q1>>8)   (both < 2^23, exact)
    ah = tmp.tile([P, n], I32, tag=f"ah_{n}")
    bh = tmp.tile([P, n], I32, tag=f"bh_{n}")
    t1 = tmp.tile([P, n], I32, tag=f"t1_{n}")
    nc.vector.tensor_single_scalar(t1[:], p1[:], 8, op=AL.logical_shift_right)
    nc.vector.tensor_add(ah[:], t1[:], p2[:])
    nc.vector.tensor_single_scalar(t1[:], q1[:], 8, op=AL.logical_shift_right)
    nc.vector.tensor_add(bh[:], t1[:], q2[:])
    # X>>8 and X low byte (in low 8 bits of xl)
    xh = tmp.tile([P, n], I32, tag=f"xh_{n}")
    xl = tmp.tile([P, n], I32, tag=f"xl_{n}")
    nc.vector.tensor_tensor(xh[:], ah[:], bh[:], op=AL.bitwise_xor)
    nc.vector.tensor_tensor(xl[:], p1[:], q1[:], op=AL.bitwise_xor)

    # y = (xh>>13)*8396 + ((xh & 8191) << 8) + (xl & 255)   ( < 2^24 )
    w1 = tmp.tile([P, n], I32, tag=f"w1_{n}")
    w2 = tmp.tile([P, n], I32, tag=f"w2_{n}")
    nc.vector.tensor_single_scalar(w1[:], xh[:], 13, op=AL.logical_shift_right)
    nc.vector.tensor_scalar_mul(w1[:], w1[:], float(C21))
    nc.vector.tensor_scalar(w2[:], xh[:], 8191, 8,
                            op0=AL.bitwise_and, op1=AL.logical_shift_left)
    w3 = tmp.tile([P, n], I32, tag=f"w3_{n}")
    nc.vector.tensor_add(w3[:], w1[:], w2[:])
    y = tmp.tile([P, n], I32, tag=f"y_{n}")
    nc.vector.tensor_single_scalar(y[:], xl[:], 255, op=AL.bitwise_and)
    nc.vector.tensor_add(y[:], y[:], w3[:])

    # r = y - rne(y/m)*m  (HW converter is round-to-nearest => r < m always)
    qt = tmp.tile([P, n], I32, tag=f"qt_{n}")
    _mul(nc, qt[:], y[:], INV_M)
    r = tmp.tile([P, n], I32, tag=f"r_{n}")
    nc.vector.scalar_tensor_tensor(r[:], qt[:], -float(MOD), y[:],
                                   op0=AL.mult, op1=AL.add)
    if SIM_COMPAT:
        f1 = tmp.tile([P, n], I32, tag=f"f1_{n}")
        nc.vector.tensor_single_scalar(f1[:], r[:], float(MOD), op=AL.is_ge)
        nc.vector.scalar_tensor_tensor(r[:], f1[:], -float(MOD), r[:],
                                       op0=AL.mult, op1=AL.add)
    f2 = tmp.tile([P, n], I32, tag=f"f2_{n}")
    nc.vector.tensor_single_scalar(f2[:], r[:], 0.0, op=AL.is_lt)
    nc.vector.scalar_tensor_tensor(r[:], f2[:], float(MOD), r[:],
                                   op0=AL.mult, op1=AL.add)

    if cs == 0:
        # token 0 (partition p%16==0, col 0): h = MOD
        nc.vector.tensor_mul(r[:, 0:1], r[:, 0:1], mask[:])
        nc.vector.tensor_add(r[:, 0:1], r[:, 0:1], offs[:])

    nc.vector.tensor_copy(idx[:, cs:cs + n], r[:])


def body(ctx: ExitStack, tc: tile.TileContext, out_ap, tok_ap, table_ap,
         proj_ap, scale_ap, dbg=None):
    """Emit the per-core kernel.  tok_ap is int32 [2, 16, SPT]: the host-
    staged 16-wrap current-token and previous-token tiles."""
    nc = tc.nc

    const = ctx.enter_context(tc.tile_pool(name="const", bufs=1))
    tmp = ctx.enter_context(tc.tile_pool(name="tmp", bufs=1))
    gpool = ctx.enter_context(tc.tile_pool(name="gpool", bufs=1))
    et_pool = ctx.enter_context(tc.tile_pool(name="et", bufs=6))
    o_pool = ctx.enter_context(tc.tile_pool(name="osb", bufs=3))

    # ---- tokens: host-staged 16-wrap tiles, contiguous per-partition,
    # replicated x8 by partition-doubling (16->32->64->128, two parallel
    # DMA chains on SP and ACT) ----
    cur = const.tile([P, SPT], I32)
    prv = const.tile([P, SPT], I32)
    nc.sync.dma_start(cur[0:16, :], tok_ap[0])
    nc.scalar.dma_start(prv[0:16, :], tok_ap[1])
    for w in (16, 32, 64):
        nc.sync.dma_start(cur[w:2 * w, :], cur[0:w, :])
        nc.scalar.dma_start(prv[w:2 * w, :], prv[0:w, :])

    # partition masks for the token-0 override
    pi = const.tile([P, 1], I32)
    nc.gpsimd.iota(pi[:], pattern=[[0, 1]], base=0, channel_multiplier=1)
    mask = const.tile([P, 1], I32)
    nc.vector.tensor_single_scalar(mask[:], pi[:], 15, op=AL.bitwise_and)
    nc.vector.tensor_single_scalar(mask[:], mask[:], 0.0, op=AL.not_equal)
    offs = const.tile([P, 1], I32)
    nc.vector.tensor_scalar(offs[:], mask[:], -float(MOD), float(MOD),
                            op0=AL.mult, op1=AL.add)

    idx = const.tile([P, SPT], I16)
    # g_sb[p, b, :] = embed_f32[h(token 128b + p), :] -- gathered straight
    # from the fp32 table (512B rows cost the same DMA descriptor time as
    # 256B ones, and skipping the bf16 pre-conversion removes a 7.9 MB
    # DMA flood that otherwise stalls startup by ~25 us).  The bf16 cast
    # happens for free in the eT PSUM->SBUF copy.
    g_sb = gpool.tile([P, NB, P], F32)

    # hash + gathers (each chunk covers whole gathers; gather = CPG columns)
    swdge_i = 0
    cs = 0
    for n in HASH_CHUNKS:
        _hash_chunk(nc, tmp, idx, cur, prv, mask, offs, cs, n)
        for g in range(cs // CPG, (cs + n) // CPG):
            nc.gpsimd.dma_gather(
                g_sb[:, BPG * g:BPG * (g + 1), :],
                table_ap,
                idx[:, CPG * g:CPG * (g + 1)],
                num_idxs=IPG,
                num_idxs_reg=IPG,
                elem_size=D,
                single_packet=False,
                queue_num=swdge_i % N_QUEUES,
            )
            swdge_i += 1
        cs += n

    if dbg is not None:
        nc.sync.dma_start(dbg["idx"], idx[:])
        nc.sync.dma_start(dbg["cur"], cur[:])
        nc.sync.dma_start(dbg["prv"], prv[:])

    # ---- setup: identity, projT (transposed, pre-scaled, bf16) ----
    ps_setup = tc.alloc_tile_pool(name="ps_setup", bufs=1, space="PSUM")
    ident_f = const.tile([P, P], F32)
    make_identity(nc, ident_f[:])

    # scale broadcast [1,1] -> [128,1] via K=1 matmul with a ones row
    sc_in = const.tile([1, 1], F32)
    nc.sync.dma_start(sc_in[:], scale_ap)
    ones = const.tile([1, P], F32)
    nc.gpsimd.memset(ones[:], 1.0)
    ps_sc = ps_setup.tile([P, 1], F32, space="PSUM", tag="ps_sc")
    nc.tensor.matmul(ps_sc[:], lhsT=ones[:], rhs=sc_in[:], start=True, stop=True)
    sc_b = const.tile([P, 1], F32)
    nc.vector.tensor_copy(sc_b[:], ps_sc[:])

    projT = const.tile([P, M], F32)
    for c in range(M // P):
        pch = tmp.tile([P, P], F32, tag="pch")
        nc.sync.dma_start(pch[:], proj_ap[c * P:(c + 1) * P, :])
        ps_t = ps_setup.tile([P, P], F32, space="PSUM", tag="ps_t")
        nc.tensor.transpose(ps_t[:], pch[:], ident_f[:])
        nc.vector.tensor_copy(projT[:, c * P:(c + 1) * P], ps_t[:])
    nc.vector.tensor_scalar_mul(projT[:], projT[:], sc_b[:, 0:1])
    projT_b = const.tile([P, M], BF16)
    nc.vector.tensor_copy(projT_b[:], projT[:])
    ps_setup.release()

    ps_small = ctx.enter_context(tc.tile_pool(name="ps_small", bufs=4, space="PSUM"))
    ps_big = ctx.enter_context(tc.tile_pool(name="ps_big", bufs=4, space="PSUM"))

    # main loop.  Block b holds tokens 128b..128b+127 in order, so matmul
    # outputs are token-contiguous and each GRP-block DMA writes one
    # sequential 256KB DRAM run.  eT and out copies alternate DVE/ACT.
    ets = {}
    o2s = {}

    def emit_trans(b):
        ps_et = ps_small.tile([P, P], F32, space="PSUM",
                              tag="ps_et", name=f"ps_et{b}")
        nc.tensor.transpose(ps_et[:], g_sb[:, b, :], ident_f[:])
        et = et_pool.tile([P, P], BF16, tag="et", name=f"et{b}")
        if b % 2:
            nc.scalar.copy(et[:], ps_et[:])
        else:
            nc.vector.tensor_copy(et[:], ps_et[:])
        ets[b] = et

    def emit_mm(b):
        et = ets.pop(b)
        gi, gb = divmod(b, GRP)
        if gb == 0:
            o2s[gi] = o_pool.tile([P, GRP, M], BF16, tag="o_sb",
                                  name=f"o2_{gi}")
        o2 = o2s[gi]
        ps_o = ps_big.tile([P, M], F32, space="PSUM", tag="ps_o",
                           name=f"ps_o{b}")
        nc.tensor.matmul(ps_o[:], lhsT=et[:], rhs=projT_b[:],
                         start=True, stop=True)
        if b % 2:
            nc.vector.tensor_copy(o2[:, gb, :], ps_o[:])
        else:
            nc.scalar.copy(o2[:, gb, :], ps_o[:])
        if gb == GRP - 1:
            # tokens GRP*128*gi .. +GRP*128: [128, GRP, M] -> rows
            # (gb*128 + p) of the GRP*128-token range, partition-major
            dst = out_ap[GRP * P * gi:GRP * P * (gi + 1), :]
            dst = dst.rearrange("(g p) m -> p g m", g=GRP)
            nc.sync.dma_start(dst, o2[:])
            del o2s[gi]

    for b in range(NB):
        emit_trans(b)
        if b >= LAG:
            emit_mm(b - LAG)
    for b in range(NB - LAG, NB):
        emit_mm(b)


_CACHE: dict = {}
DEBUG = False    # dump idx/cur/prv tiles to DRAM for stage checking


def _build(key: int = 0):
    if key in _CACHE:
        return _CACHE[key]
    nc = bacc.Bacc("TRN2", target_bir_lowering=False, debug=False,
                   num_swdge_queues=N_QUEUES, dynamic_dma_scratch_size=131072)
    tok = nc.dram_tensor("token_ids", [2, 16, SPT], I32, kind="ExternalInput").ap()
    table = nc.dram_tensor("embed_weight", [V, D], F32, kind="ExternalInput").ap()
    proj = nc.dram_tensor("proj_weight", [M, D], F32, kind="ExternalInput").ap()
    scale = nc.dram_tensor("scale", [1, 1], F32, kind="ExternalInput").ap()
    out = nc.dram_tensor("out", [S, M], BF16, kind="ExternalOutput").ap()
    dbg = None
    if DEBUG:
        dbg = {
            "idx": nc.dram_tensor("idx_dbg", [P, SPT], I16,
                                  kind="ExternalOutput").ap(),
            "cur": nc.dram_tensor("cur_dbg", [P, SPT], I32,
                                  kind="ExternalOutput").ap(),
            "prv": nc.dram_tensor("prv_dbg", [P, SPT], I32,
                                  kind="ExternalOutput").ap(),
        }
    with tile.TileContext(nc) as tc:
        with ExitStack() as ctx:
            body(ctx, tc, out, tok, table, proj, scale, dbg=dbg)
    nc.compile()
    _CACHE[key] = nc
    return nc


def stage_tokens(row: np.ndarray) -> np.ndarray:
    """[S] int token row -> [2, 16, SPT] int32 16-wrap (cur, prev) tiles."""
    t32 = row.astype(np.int32)          # values < 2^31; lo-word == value
    prev = np.empty_like(t32)
    prev[0] = 0
    prev[1:] = t32[:-1]
    cur_w = np.ascontiguousarray(t32.reshape(SPT, 16).T)
    prv_w = np.ascontiguousarray(prev.reshape(SPT, 16).T)
    return np.ascontiguousarray(np.stack([cur_w, prv_w]))


def kernel(token_ids: np.ndarray, embed_weight: np.ndarray,
           proj_weight: np.ndarray, scale: np.ndarray) -> np.ndarray:
    token_ids = np.ascontiguousarray(token_ids)
    assert token_ids.shape == (B, S), token_ids.shape
    table = np.ascontiguousarray(embed_weight, dtype=np.float32)
    proj = np.ascontiguousarray(proj_weight, dtype=np.float32)
    sc = np.asarray(scale, dtype=np.float32).reshape(1, 1)

    nc = _build()
    in_maps = [
        {
            "token_ids": stage_tokens(token_ids[i]),
            "embed_weight": table,
            "proj_weight": proj,
            "scale": sc,
        }
        for i in range(B)
    ]
    res = run_bass_kernel_spmd(nc, in_maps, core_ids=list(range(B)))
    return np.stack([np.asarray(r["out"]).astype(np.float32)
                     for r in res.results], axis=0)


# revision 21
# speedup vs baseline: 1.3830x; 1.0110x over previous
"""Trainium2 Bass kernel: BigramHashEmbedding (hash -> embed gather -> proj -> scale).

Computation (per batch row, one NeuronCore per row, 8 rows total):
    h[0]  = 10239
    h[j]  = (36313*t[j] ^ 27191*t[j-1]) % 10239          (int32, j >= 1)
    e     = embed_weight[h]                               [S, 128] gather
    out   = (e @ proj_weight.T) * scale                   [S, 512]

Device strategy per core (S = 8192 tokens):
  * dma_gather unwraps its index tile column-major over 16 partitions
    (slot k <- idx[k%16, k//16]), so the host stages tokens in a 16-wrap
    layout (tok16[p, s] = t[16s + p], plus a one-shifted copy for the
    bigram's previous token; both are pure permutations of the int32 index
    tensor, staged as one [2, 16, 512] input).  With this layout gather
    slot k maps to token k exactly: gathered rows land as
    g_sb[p, b, :] = e[token 128b + p], the PE transpose of block b yields
    eT in plain token order, and every output DMA writes 128 contiguous
    rows (fully sequential HBM addresses).
  * the device loads the two wrapped tiles with contiguous 2KB-per-
    partition runs and broadcasts them x8 across the 128 partitions (the
    gather needs its idx rows replicated per GpSimd core pair; the hash
    then runs on all 128 DVE lanes).
  * the bigram hash runs on DVE/ACT with fp32-exact arithmetic: products
    are split (36313 = 141*256 + 217, 27191 = 106*256 + 55) so every
    arithmetic op stays below 2^24 (the vector ALU is fp32 internally);
    >=2^24 values only pass through bitwise ops, which are bit-exact.
    mod-10239 is a limb decomposition X = u*2^21 + v*2^8 + w ->
    y = u*8396 + (v<<8) + w (y < 2^24) plus one fp32 reciprocal-multiply
    quotient; the HW float->int converter rounds to nearest, so a single
    +m fixup suffices.
  * the embed table is converted once to bf16 in DRAM (cast-during-DMA on
    SWDGE, split into 4 queue-parallel chunks to shorten the startup
    serialization).  Eight dma_gathers (1024 rows each) fetch rows into
    [128, 64, 128] bf16.  (The transpose=True gather mode would skip the
    PE transposes below, but it routes through the shared XBAR: its
    descriptor generation costs ~8.5 ns/row and concurrent transposed
    gathers on different queues corrupt each other, so serialized it is
    ~70 us for 8K rows -- measured.  Plain gathers + PE transposes win.)
  * per 128-token block: bf16 PE transpose (identity) -> PSUM -> bf16 eT
    in SBUF (DVE/ACT alternating copy), then PE matmul eT.T @ projT_bf16
    -> PSUM f32 -> bf16 copy into a 2-block SBUF group (DVE/ACT
    alternating) -> one HWDGE DMA per 2 blocks (256KB contiguous).  The
    output tensor is bf16; the host upcasts to f32 (tolerance ~2e-2, bf16
    out adds ~2e-3).  Transposes run LAG blocks ahead of the matmuls so
    the eT copy stays off the PE's in-order critical path.
  * proj [512, 128] is transposed on the PE at setup into projT [128,
    512], pre-scaled by `scale` (broadcast via a K=1 matmul), cast bf16.

SWDGE semaphore lanes are round-robin (8) and lock to one queue each, so
every SWDGE DMA uses queue = emission_index % N_QUEUES to keep lane->queue
stable across the wrap (12 SWDGE DMAs: 4 conversion chunks + 8 gathers).
"""

from contextlib import ExitStack

import numpy as np

import concourse.bacc as bacc
import concourse.bass as bass
import concourse.mybir as mybir
import concourse.tile as tile
from concourse.bass_utils import run_bass_kernel_spmd
from concourse.masks import make_identity

AL = mybir.AluOpType
F32 = mybir.dt.float32
BF16 = mybir.dt.bfloat16
I32 = mybir.dt.int32
I16 = mybir.dt.int16

B = 8           # batch rows == cores
S = 8192        # tokens per core
V = 10240       # hash table rows
D = 128         # embed dim
M = 512         # model dim
P = 128
MOD = 10239     # hash modulus (HASH_SIZE - 1)
SPT = S // 16   # 16-wrap columns = 512
NG = 8          # gathers
IPG = S // NG   # idxs per gather = 1024
CPG = IPG // 16  # idx columns per gather = 64
NB = S // P     # 128-token blocks = 64
BPG = IPG // P  # matmul blocks per gather = 8
NCONV = 4       # table-conversion chunks
HASH_CHUNKS = (64, 64, 128, 256)   # progressive: short first chain, wide later
assert sum(HASH_CHUNKS) == SPT

# 36313 = 141*256 + 217 ; 27191 = 106*256 + 55
A_HI, A_LO = 141, 217
B_HI, B_LO = 106, 55
C21 = 8396      # 2^21 mod 10239
INV_M = 1.0 / MOD

USE_ACT_MUL = True   # run the big hash multiplies on the Scalar (ACT) engine
N_QUEUES = 4         # SWDGE queues (ucode MAX_SWDGE_QUEUES=4)
SIM_COMPAT = False   # add the >=MOD fixup (only needed under CoreSim's trunc convert)
LAG = 4              # transpose runs LAG blocks ahead of the matmul
GRP = 2              # output blocks per DMA (256KB contiguous)


def _mul(nc, out, in_, const):
    if USE_ACT_MUL:
        nc.scalar.mul(out, in_, float(const))
    else:
        nc.vector.tensor_scalar_mul(out, in_, float(const))


def _hash_chunk(nc, tmp, idx, cur, prv, mask, offs, cs, n):
    """Emit ops computing idx[:, cs:cs+n] (int16 hash values).

    cur: [128, SPT] int32, cur[p, s] = t[16s + p%16]   (x8 replicas)
    prv: [128, SPT] int32, prv[p, s] = t[16s + p%16 - 1] (0 at (p%16==0, 0))
    mask: [128, 1] int32, (p % 16) != 0.
    offs: [128, 1] int32, 10239 * (p % 16 == 0).
    """
    tcur = cur[:, cs:cs + n]
    tprev = prv[:, cs:cs + n]
    p1 = tmp.tile([P, n], I32, tag=f"p1_{n}")
    p2 = tmp.tile([P, n], I32, tag=f"p2_{n}")
    q1 = tmp.tile([P, n], I32, tag=f"q1_{n}")
    q2 = tmp.tile([P, n], I32, tag=f"q2_{n}")
    _mul(nc, p1[:], tcur, A_LO)
    _mul(nc, p2[:], tcur, A_HI)
    _mul(nc, q1[:], tprev, B_LO)
    _mul(nc, q2[:], tprev, B_HI)

    # A>>8 = p2 + (p1>>8);  B>>8 = q2 + (q1>>8)   (both < 2^23, exact)
    ah = tmp.tile([P, n], I32, tag=f"ah_{n}")
    bh = tmp.tile([P, n], I32, tag=f"bh_{n}")
    t1 = tmp.tile([P, n], I32, tag=f"t1_{n}")
    nc.vector.tensor_single_scalar(t1[:], p1[:], 8, op=AL.logical_shift_right)
    nc.vector.tensor_add(ah[:], t1[:], p2[:])
    nc.vector.tensor_single_scalar(t1[:], q1[:], 8, op=AL.logical_shift_right)
    nc.vector.tensor_add(bh[:], t1[:], q2[:])
    # X>>8 and X low byte (in low 8 bits of xl)
    xh = tmp.tile([P, n], I32, tag=f"xh_{n}")
    xl = tmp.tile([P, n], I32, tag=f"xl_{n}")
    nc.vector.tensor_tensor(xh[:], ah[:], bh[:], op=AL.bitwise_xor)
    nc.vector.tensor_tensor(xl[:], p1[:], q1[:], op=AL.bitwise_xor)

    # y = (xh>>13)*8396 + ((xh & 8191) << 8) + (xl & 255)   ( < 2^24 )
    w1 = tmp.tile([P, n], I32, tag=f"w1_{n}")
    w2 = tmp.tile([P, n], I32, tag=f"w2_{n}")
    nc.vector.tensor_single_scalar(w1[:], xh[:], 13, op=AL.logical_shift_right)
    nc.vector.tensor_scalar_mul(w1[:], w1[:], float(C21))
    nc.vector.tensor_scalar(w2[:], xh[:], 8191, 8,
                            op0=AL.bitwise_and, op1=AL.logical_shift_left)
    w3 = tmp.tile([P, n], I32, tag=f"w3_{n}")
    nc.vector.tensor_add(w3[:], w1[:], w2[:])
    y = tmp.tile([P, n], I32, tag=f"y_{n}")
    nc.vector.tensor_single_scalar(y[:], xl[:], 255, op=AL.bitwise_and)
    nc.vector.tensor_add(y[:], y[:], w3[:])

    # r = y - rne(y/m)*m  (HW converter is round-to-nearest => r < m always)
    qt = tmp.tile([P, n], I32, tag=f"qt_{n}")
    _mul(nc, qt[:], y[:], INV_M)
    r = tmp.tile([P, n], I32, tag=f"r_{n}")
    nc.vector.scalar_tensor_tensor(r[:], qt[:], -float(MOD), y[:],
                                   op0=AL.mult, op1=AL.add)
    if SIM_COMPAT:
        f1 = tmp.tile([P, n], I32, tag=f"f1_{n}")
        nc.vector.tensor_single_scalar(f1[:], r[:], float(MOD), op=AL.is_ge)
        nc.vector.scalar_tensor_tensor(r[:], f1[:], -float(MOD), r[:],
                                       op0=AL.mult, op1=AL.add)
    f2 = tmp.tile([P, n], I32, tag=f"f2_{n}")
    nc.vector.tensor_single_scalar(f2[:], r[:], 0.0, op=AL.is_lt)
    nc.vector.scalar_tensor_tensor(r[:], f2[:], float(MOD), r[:],
                                   op0=AL.mult, op1=AL.add)

    if cs == 0:
        # token 0 (partition p%16==0, col 0): h = MOD
        nc.vector.tensor_mul(r[:, 0:1], r[:, 0:1], mask[:])
        nc.vector.tensor_add(r[:, 0:1], r[:, 0:1], offs[:])

    nc.vector.tensor_copy(idx[:, cs:cs + n], r[:])


def body(ctx: ExitStack, tc: tile.TileContext, out_ap, tok_ap, table_ap,
         proj_ap, scale_ap, dbg=None):
    """Emit the per-core kernel.  tok_ap is int32 [2, 16, SPT]: the host-
    staged 16-wrap current-token and previous-token tiles."""
    nc = tc.nc

    const = ctx.enter_context(tc.tile_pool(name="const", bufs=1))
    tmp = ctx.enter_context(tc.tile_pool(name="tmp", bufs=1))
    gpool = ctx.enter_context(tc.tile_pool(name="gpool", bufs=1))
    et_pool = ctx.enter_context(tc.tile_pool(name="et", bufs=6))
    o_pool = ctx.enter_context(tc.tile_pool(name="osb", bufs=3))

    # ---- tokens: host-staged 16-wrap tiles, pre-replicated x8 so a
    # single contiguous DMA per tile makes them hash-ready ----
    cur = const.tile([P, SPT], I32)
    prv = const.tile([P, SPT], I32)
    nc.sync.dma_start(cur[:], tok_ap[0])
    nc.scalar.dma_start(prv[:], tok_ap[1])

    # partition masks for the token-0 override
    pi = const.tile([P, 1], I32)
    nc.gpsimd.iota(pi[:], pattern=[[0, 1]], base=0, channel_multiplier=1)
    mask = const.tile([P, 1], I32)
    nc.vector.tensor_single_scalar(mask[:], pi[:], 15, op=AL.bitwise_and)
    nc.vector.tensor_single_scalar(mask[:], mask[:], 0.0, op=AL.not_equal)
    offs = const.tile([P, 1], I32)
    nc.vector.tensor_scalar(offs[:], mask[:], -float(MOD), float(MOD),
                            op0=AL.mult, op1=AL.add)

    idx = const.tile([P, SPT], I16)
    # g_sb[p, b, :] = embed_f32[h(token 128b + p), :] -- gathered straight
    # from the fp32 table (512B rows cost the same DMA descriptor time as
    # 256B ones, and skipping the bf16 pre-conversion removes a 7.9 MB
    # DMA flood that otherwise stalls startup by ~25 us).  The bf16 cast
    # happens for free in the eT PSUM->SBUF copy.
    g_sb = gpool.tile([P, NB, P], F32)

    # hash + gathers (each chunk covers whole gathers; gather = CPG columns)
    swdge_i = 0
    cs = 0
    for n in HASH_CHUNKS:
        _hash_chunk(nc, tmp, idx, cur, prv, mask, offs, cs, n)
        for g in range(cs // CPG, (cs + n) // CPG):
            nc.gpsimd.dma_gather(
                g_sb[:, BPG * g:BPG * (g + 1), :],
                table_ap,
                idx[:, CPG * g:CPG * (g + 1)],
                num_idxs=IPG,
                num_idxs_reg=IPG,
                elem_size=D,
                single_packet=False,
                queue_num=swdge_i % N_QUEUES,
            )
            swdge_i += 1
        cs += n

    if dbg is not None:
        nc.sync.dma_start(dbg["idx"], idx[:])
        nc.sync.dma_start(dbg["cur"], cur[:])
        nc.sync.dma_start(dbg["prv"], prv[:])

    # ---- setup: identity, projT (transposed, pre-scaled, bf16) ----
    ps_setup = tc.alloc_tile_pool(name="ps_setup", bufs=1, space="PSUM")
    ident_f = const.tile([P, P], F32)
    make_identity(nc, ident_f[:])

    # scale broadcast [1,1] -> [128,1] via K=1 matmul with a ones row
    sc_in = const.tile([1, 1], F32)
    nc.sync.dma_start(sc_in[:], scale_ap)
    ones = const.tile([1, P], F32)
    nc.gpsimd.memset(ones[:], 1.0)
    ps_sc = ps_setup.tile([P, 1], F32, space="PSUM", tag="ps_sc")
    nc.tensor.matmul(ps_sc[:], lhsT=ones[:], rhs=sc_in[:], start=True, stop=True)
    sc_b = const.tile([P, 1], F32)
    nc.vector.tensor_copy(sc_b[:], ps_sc[:])

    projT = const.tile([P, M], F32)
    for c in range(M // P):
        pch = tmp.tile([P, P], F32, tag="pch")
        nc.sync.dma_start(pch[:], proj_ap[c * P:(c + 1) * P, :])
        ps_t = ps_setup.tile([P, P], F32, space="PSUM", tag="ps_t")
        nc.tensor.transpose(ps_t[:], pch[:], ident_f[:])
        nc.vector.tensor_copy(projT[:, c * P:(c + 1) * P], ps_t[:])
    nc.vector.tensor_scalar_mul(projT[:], projT[:], sc_b[:, 0:1])
    projT_b = const.tile([P, M], BF16)
    nc.vector.tensor_copy(projT_b[:], projT[:])
    ps_setup.release()

    ps_small = ctx.enter_context(tc.tile_pool(name="ps_small", bufs=2, space="PSUM"))
    ps_big = ctx.enter_context(tc.tile_pool(name="ps_big", bufs=3, space="PSUM"))

    # main loop, processed in PAIRS of 128-token blocks to halve the
    # per-stage semaphore handoffs (which otherwise latency-bind the PE
    # phase): two transposes share a PSUM tile and one eT copy; two
    # matmuls share a 2-bank PSUM tile, one bf16 copy and one 256KB
    # contiguous DMA.  Copies alternate DVE/ACT.  Block b holds tokens
    # 128b..128b+127 in order, so all output DMAs are sequential.
    NPAIR = NB // 2
    ets = {}

    def emit_trans(pb):
        ps_et = ps_small.tile([P, 2, P], F32, space="PSUM",
                              tag="ps_et", name=f"ps_et{pb}")
        nc.tensor.transpose(ps_et[:, 0, :], g_sb[:, 2 * pb, :], ident_f[:])
        nc.tensor.transpose(ps_et[:, 1, :], g_sb[:, 2 * pb + 1, :], ident_f[:])
        et = et_pool.tile([P, 2, P], BF16, tag="et", name=f"et{pb}")
        if pb % 2:
            nc.scalar.copy(et[:], ps_et[:])
        else:
            nc.vector.tensor_copy(et[:], ps_et[:])
        ets[pb] = et

    def emit_mm(pb):
        et = ets.pop(pb)
        ps_o = ps_big.tile([P, 2, M], F32, space="PSUM", tag="ps_o",
                           name=f"ps_o{pb}")
        nc.tensor.matmul(ps_o[:, 0, :], lhsT=et[:, 0, :], rhs=projT_b[:],
                         start=True, stop=True)
        nc.tensor.matmul(ps_o[:, 1, :], lhsT=et[:, 1, :], rhs=projT_b[:],
                         start=True, stop=True)
        o2 = o_pool.tile([P, 2, M], BF16, tag="o_sb", name=f"o2_{pb}")
        if pb % 2:
            nc.vector.tensor_copy(o2[:], ps_o[:])
        else:
            nc.scalar.copy(o2[:], ps_o[:])
        dst = out_ap[2 * P * pb:2 * P * (pb + 1), :]
        dst = dst.rearrange("(g p) m -> p g m", g=2)
        nc.sync.dma_start(dst, o2[:])

    for pb in range(NPAIR):
        emit_trans(pb)
        if pb >= LAG:
            emit_mm(pb - LAG)
    for pb in range(NPAIR - LAG, NPAIR):
        emit_mm(pb)


_CACHE: dict = {}
DEBUG = False    # dump idx/cur/prv tiles to DRAM for stage checking


def _build(key: int = 0):
    if key in _CACHE:
        return _CACHE[key]
    nc = bacc.Bacc("TRN2", target_bir_lowering=False, debug=False,
                   num_swdge_queues=N_QUEUES, dynamic_dma_scratch_size=131072)
    tok = nc.dram_tensor("token_ids", [2, P, SPT], I32, kind="ExternalInput").ap()
    table = nc.dram_tensor("embed_weight", [V, D], F32, kind="ExternalInput").ap()
    proj = nc.dram_tensor("proj_weight", [M, D], F32, kind="ExternalInput").ap()
    scale = nc.dram_tensor("scale", [1, 1], F32, kind="ExternalInput").ap()
    out = nc.dram_tensor("out", [S, M], BF16, kind="ExternalOutput").ap()
    dbg = None
    if DEBUG:
        dbg = {
            "idx": nc.dram_tensor("idx_dbg", [P, SPT], I16,
                                  kind="ExternalOutput").ap(),
            "cur": nc.dram_tensor("cur_dbg", [P, SPT], I32,
                                  kind="ExternalOutput").ap(),
            "prv": nc.dram_tensor("prv_dbg", [P, SPT], I32,
                                  kind="ExternalOutput").ap(),
        }
    with tile.TileContext(nc) as tc:
        with ExitStack() as ctx:
            body(ctx, tc, out, tok, table, proj, scale, dbg=dbg)
    nc.compile()
    _CACHE[key] = nc
    return nc


def stage_tokens(row: np.ndarray) -> np.ndarray:
    """[S] int token row -> [2, 128, SPT] int32 16-wrap (cur, prev) tiles,
    pre-replicated x8 across the partition dim (the gather requires its idx
    rows replicated per GpSimd core pair, and the hash then uses all 128
    DVE lanes)."""
    t32 = row.astype(np.int32)          # values < 2^31; lo-word == value
    prev = np.empty_like(t32)
    prev[0] = 0
    prev[1:] = t32[:-1]
    cur_w = np.tile(t32.reshape(SPT, 16).T, (8, 1))
    prv_w = np.tile(prev.reshape(SPT, 16).T, (8, 1))
    return np.ascontiguousarray(np.stack([cur_w, prv_w]))


def kernel(token_ids: np.ndarray, embed_weight: np.ndarray,
           proj_weight: np.ndarray, scale: np.ndarray) -> np.ndarray:
    token_ids = np.ascontiguousarray(token_ids)
    assert token_ids.shape == (B, S), token_ids.shape
    table = np.ascontiguousarray(embed_weight, dtype=np.float32)
    proj = np.ascontiguousarray(proj_weight, dtype=np.float32)
    sc = np.asarray(scale, dtype=np.float32).reshape(1, 1)

    nc = _build()
    in_maps = [
        {
            "token_ids": stage_tokens(token_ids[i]),
            "embed_weight": table,
            "proj_weight": proj,
            "scale": sc,
        }
        for i in range(B)
    ]
    res = run_bass_kernel_spmd(nc, in_maps, core_ids=list(range(B)))
    return np.stack([np.asarray(r["out"]).astype(np.float32)
                     for r in res.results], axis=0)


# revision 30
# speedup vs baseline: 1.4249x; 1.0303x over previous
"""Trainium2 Bass kernel: BigramHashEmbedding (hash -> embed gather -> proj -> scale).

Computation (per batch row, one NeuronCore per row, 8 rows total):
    h[0]  = 10239
    h[j]  = (36313*t[j] ^ 27191*t[j-1]) % 10239          (int32, j >= 1)
    e     = embed_weight[h]                               [S, 128] gather
    out   = (e @ proj_weight.T) * scale                   [S, 512]

Device strategy per core (S = 8192 tokens):
  * dma_gather unwraps its index tile column-major over 16 partitions
    (slot k <- idx[k%16, k//16]), so the host stages tokens in a 16-wrap
    layout (tok16[p, s] = t[16s + p], plus a one-shifted copy for the
    bigram's previous token; both are pure permutations of the int32 index
    tensor, staged as one [2, 16, 512] input).  With this layout gather
    slot k maps to token k exactly: gathered rows land as
    g_sb[p, b, :] = e[token 128b + p], the PE transpose of block b yields
    eT in plain token order, and every output DMA writes 128 contiguous
    rows (fully sequential HBM addresses).
  * the device loads the two wrapped tiles with contiguous 2KB-per-
    partition runs and broadcasts them x8 across the 128 partitions (the
    gather needs its idx rows replicated per GpSimd core pair; the hash
    then runs on all 128 DVE lanes).
  * the bigram hash runs on DVE/ACT with fp32-exact arithmetic: products
    are split (36313 = 141*256 + 217, 27191 = 106*256 + 55) so every
    arithmetic op stays below 2^24 (the vector ALU is fp32 internally);
    >=2^24 values only pass through bitwise ops, which are bit-exact.
    mod-10239 is a limb decomposition X = u*2^21 + v*2^8 + w ->
    y = u*8396 + (v<<8) + w (y < 2^24) plus one fp32 reciprocal-multiply
    quotient; the HW float->int converter rounds to nearest, so a single
    +m fixup suffices.
  * the embed table is converted once to bf16 in DRAM (cast-during-DMA on
    SWDGE, split into 4 queue-parallel chunks to shorten the startup
    serialization).  Eight dma_gathers (1024 rows each) fetch rows into
    [128, 64, 128] bf16.  (The transpose=True gather mode would skip the
    PE transposes below, but it routes through the shared XBAR: its
    descriptor generation costs ~8.5 ns/row and concurrent transposed
    gathers on different queues corrupt each other, so serialized it is
    ~70 us for 8K rows -- measured.  Plain gathers + PE transposes win.)
  * per 128-token block: bf16 PE transpose (identity) -> PSUM -> bf16 eT
    in SBUF (DVE/ACT alternating copy), then PE matmul eT.T @ projT_bf16
    -> PSUM f32 -> bf16 copy into a 2-block SBUF group (DVE/ACT
    alternating) -> one HWDGE DMA per 2 blocks (256KB contiguous).  The
    output tensor is bf16; the host upcasts to f32 (tolerance ~2e-2, bf16
    out adds ~2e-3).  Transposes run LAG blocks ahead of the matmuls so
    the eT copy stays off the PE's in-order critical path.
  * proj [512, 128] is transposed on the PE at setup into projT [128,
    512], pre-scaled by `scale` (broadcast via a K=1 matmul), cast bf16.

SWDGE semaphore lanes are round-robin (8) and lock to one queue each, so
every SWDGE DMA uses queue = emission_index % N_QUEUES to keep lane->queue
stable across the wrap (12 SWDGE DMAs: 4 conversion chunks + 8 gathers).
"""

from contextlib import ExitStack

import numpy as np

import concourse.bacc as bacc
import concourse.bass as bass
import concourse.mybir as mybir
import concourse.tile as tile
from concourse.bass_utils import run_bass_kernel_spmd
from concourse.masks import make_identity

AL = mybir.AluOpType
F32 = mybir.dt.float32
BF16 = mybir.dt.bfloat16
I32 = mybir.dt.int32
I16 = mybir.dt.int16

B = 8           # batch rows == cores
S = 8192        # tokens per core
V = 10240       # hash table rows
D = 128         # embed dim
M = 512         # model dim
P = 128
MOD = 10239     # hash modulus (HASH_SIZE - 1)
SPT = S // 16   # 16-wrap columns = 512
NG = 8          # gathers
IPG = S // NG   # idxs per gather = 1024
CPG = IPG // 16  # idx columns per gather = 64
NB = S // P     # 128-token blocks = 64
BPG = IPG // P  # matmul blocks per gather = 8
HASH_CHUNKS = (256, 256)   # two wide chunks: ACT muls of c2 overlap DVE c1
assert sum(HASH_CHUNKS) == SPT

# 36313 = 141*256 + 217 ; 27191 = 106*256 + 55
A_HI, A_LO = 141, 217
B_HI, B_LO = 106, 55
C21 = 8396      # 2^21 mod 10239
INV_M = 1.0 / MOD

USE_ACT_MUL = True   # run the big hash multiplies on the Scalar (ACT) engine
N_QUEUES = 4         # SWDGE queues (ucode MAX_SWDGE_QUEUES=4)
SIM_COMPAT = False   # add the >=MOD fixup (only needed under CoreSim's trunc convert)
LAG = 6              # transpose runs LAG pairs ahead of the matmul


def _mul(nc, out, in_, const):
    if USE_ACT_MUL:
        nc.scalar.mul(out, in_, float(const))
    else:
        nc.vector.tensor_scalar_mul(out, in_, float(const))


def _hash_chunk(nc, tmp, idx, cur, prv, mask, offs, cs, n):
    """Emit ops computing idx[:, cs:cs+n] (int16 hash values).

    cur: [128, SPT] int32, cur[p, s] = t[16s + p%16]   (x8 replicas)
    prv: [128, SPT] int32, prv[p, s] = t[16s + p%16 - 1] (0 at (p%16==0, 0))
    mask: [128, 1] int32, (p % 16) != 0.
    offs: [128, 1] int32, 10239 * (p % 16 == 0).
    """
    tcur = cur[:, cs:cs + n]
    tprev = prv[:, cs:cs + n]
    p1 = tmp.tile([P, n], I32, tag=f"p1_{n}")
    p2 = tmp.tile([P, n], I32, tag=f"p2_{n}")
    q1 = tmp.tile([P, n], I32, tag=f"q1_{n}")
    q2 = tmp.tile([P, n], I32, tag=f"q2_{n}")
    _mul(nc, p1[:], tcur, A_LO)
    _mul(nc, p2[:], tcur, A_HI)
    _mul(nc, q1[:], tprev, B_LO)
    _mul(nc, q2[:], tprev, B_HI)

    # A>>8 = p2 + (p1>>8);  B>>8 = q2 + (q1>>8)   (both < 2^23, exact)
    ah = tmp.tile([P, n], I32, tag=f"ah_{n}")
    bh = tmp.tile([P, n], I32, tag=f"bh_{n}")
    t1 = tmp.tile([P, n], I32, tag=f"t1_{n}")
    nc.vector.tensor_single_scalar(t1[:], p1[:], 8, op=AL.logical_shift_right)
    nc.vector.tensor_add(ah[:], t1[:], p2[:])
    nc.vector.tensor_single_scalar(t1[:], q1[:], 8, op=AL.logical_shift_right)
    nc.vector.tensor_add(bh[:], t1[:], q2[:])
    # X>>8 and X low byte (in low 8 bits of xl)
    xh = tmp.tile([P, n], I32, tag=f"xh_{n}")
    xl = tmp.tile([P, n], I32, tag=f"xl_{n}")
    nc.vector.tensor_tensor(xh[:], ah[:], bh[:], op=AL.bitwise_xor)
    nc.vector.tensor_tensor(xl[:], p1[:], q1[:], op=AL.bitwise_xor)

    # y = (xh>>13)*8396 + ((xh & 8191) << 8) + (xl & 255)   ( < 2^24 )
    w1 = tmp.tile([P, n], I32, tag=f"w1_{n}")
    w2 = tmp.tile([P, n], I32, tag=f"w2_{n}")
    nc.vector.tensor_single_scalar(w1[:], xh[:], 13, op=AL.logical_shift_right)
    nc.vector.tensor_scalar_mul(w1[:], w1[:], float(C21))
    nc.vector.tensor_scalar(w2[:], xh[:], 8191, 8,
                            op0=AL.bitwise_and, op1=AL.logical_shift_left)
    w3 = tmp.tile([P, n], I32, tag=f"w3_{n}")
    nc.vector.tensor_add(w3[:], w1[:], w2[:])
    y = tmp.tile([P, n], I32, tag=f"y_{n}")
    nc.vector.tensor_single_scalar(y[:], xl[:], 255, op=AL.bitwise_and)
    nc.vector.tensor_add(y[:], y[:], w3[:])

    # r = y - rne(y/m)*m  (HW converter is round-to-nearest => r < m always)
    qt = tmp.tile([P, n], I32, tag=f"qt_{n}")
    _mul(nc, qt[:], y[:], INV_M)
    r = tmp.tile([P, n], I32, tag=f"r_{n}")
    nc.vector.scalar_tensor_tensor(r[:], qt[:], -float(MOD), y[:],
                                   op0=AL.mult, op1=AL.add)
    if SIM_COMPAT:
        f1 = tmp.tile([P, n], I32, tag=f"f1_{n}")
        nc.vector.tensor_single_scalar(f1[:], r[:], float(MOD), op=AL.is_ge)
        nc.vector.scalar_tensor_tensor(r[:], f1[:], -float(MOD), r[:],
                                       op0=AL.mult, op1=AL.add)
    f2 = tmp.tile([P, n], I32, tag=f"f2_{n}")
    nc.vector.tensor_single_scalar(f2[:], r[:], 0.0, op=AL.is_lt)
    # final fixup writes straight into the int16 idx tile (cast on store)
    nc.vector.scalar_tensor_tensor(idx[:, cs:cs + n], f2[:], float(MOD),
                                   r[:], op0=AL.mult, op1=AL.add)

    if cs == 0:
        # token 0 (partition p%16==0, col 0): h = MOD
        nc.vector.tensor_mul(idx[:, 0:1], idx[:, 0:1], mask[:])
        nc.vector.tensor_add(idx[:, 0:1], idx[:, 0:1], offs[:])


def body(ctx: ExitStack, tc: tile.TileContext, out_ap, tok_ap, table_ap,
         proj_ap, scale_ap, dbg=None):
    """Emit the per-core kernel.  tok_ap is int32 [2, 16, SPT]: the host-
    staged 16-wrap current-token and previous-token tiles."""
    nc = tc.nc

    const = ctx.enter_context(tc.tile_pool(name="const", bufs=1))
    tmp = ctx.enter_context(tc.tile_pool(name="tmp", bufs=1))
    gpool = ctx.enter_context(tc.tile_pool(name="gpool", bufs=1))
    et_pool = ctx.enter_context(tc.tile_pool(name="et", bufs=8))
    o_pool = ctx.enter_context(tc.tile_pool(name="osb", bufs=3))

    # ---- tokens: host-staged 16-wrap tiles, pre-replicated x8 so a
    # single contiguous DMA per tile makes them hash-ready ----
    cur = const.tile([P, SPT], I32)
    prv = const.tile([P, SPT], I32)
    nc.sync.dma_start(cur[:], tok_ap[0])
    nc.scalar.dma_start(prv[:], tok_ap[1])

    # partition masks for the token-0 override
    pi = const.tile([P, 1], I32)
    nc.gpsimd.iota(pi[:], pattern=[[0, 1]], base=0, channel_multiplier=1)
    m32 = const.tile([P, 1], I32)
    nc.vector.tensor_single_scalar(m32[:], pi[:], 15, op=AL.bitwise_and)
    nc.vector.tensor_single_scalar(m32[:], m32[:], 0.0, op=AL.not_equal)
    mask = const.tile([P, 1], I16)
    nc.vector.tensor_copy(mask[:], m32[:])
    offs = const.tile([P, 1], I16)
    nc.vector.tensor_scalar(offs[:], m32[:], -float(MOD), float(MOD),
                            op0=AL.mult, op1=AL.add)

    idx = const.tile([P, SPT], I16)
    # g_sb[p, b, :] = embed_f32[h(token 128b + p), :] -- gathered straight
    # from the fp32 table (512B rows cost the same DMA descriptor time as
    # 256B ones, and skipping the bf16 pre-conversion removes a 7.9 MB
    # DMA flood that otherwise stalls startup by ~25 us).  The bf16 cast
    # happens for free in the eT PSUM->SBUF copy.
    g_sb = gpool.tile([P, NB, P], F32)

    # hash chunks first, all gathers after: interleaving them makes the
    # in-order Pool stream park inside long DMAGatherAnt slices while DVE
    # hash ops stall against the Q7 descriptor-gen (measured ~7us bubbles)
    cs = 0
    for n in HASH_CHUNKS:
        _hash_chunk(nc, tmp, idx, cur, prv, mask, offs, cs, n)
        cs += n
    for g in range(NG):
        nc.gpsimd.dma_gather(
            g_sb[:, BPG * g:BPG * (g + 1), :],
            table_ap,
            idx[:, CPG * g:CPG * (g + 1)],
            num_idxs=IPG,
            num_idxs_reg=IPG,
            elem_size=D,
            single_packet=False,
            queue_num=g % N_QUEUES,
        )

    if dbg is not None:
        nc.sync.dma_start(dbg["idx"], idx[:])
        nc.sync.dma_start(dbg["cur"], cur[:])
        nc.sync.dma_start(dbg["prv"], prv[:])

    # ---- setup: identity, projT (transposed, pre-scaled, bf16) ----
    ps_setup = tc.alloc_tile_pool(name="ps_setup", bufs=1, space="PSUM")
    ident_f = const.tile([P, P], F32)
    make_identity(nc, ident_f[:])

    # scale broadcast [1,1] -> [128,1] via K=1 matmul with a ones row
    sc_in = const.tile([1, 1], F32)
    nc.sync.dma_start(sc_in[:], scale_ap)
    ones = const.tile([1, P], F32)
    nc.gpsimd.memset(ones[:], 1.0)
    ps_sc = ps_setup.tile([P, 1], F32, space="PSUM", tag="ps_sc")
    nc.tensor.matmul(ps_sc[:], lhsT=ones[:], rhs=sc_in[:], start=True, stop=True)
    sc_b = const.tile([P, 1], F32)
    nc.vector.tensor_copy(sc_b[:], ps_sc[:])

    projT = const.tile([P, M], F32)
    for c in range(M // P):
        pch = tmp.tile([P, P], F32, tag="pch")
        nc.sync.dma_start(pch[:], proj_ap[c * P:(c + 1) * P, :])
        ps_t = ps_setup.tile([P, P], F32, space="PSUM", tag="ps_t")
        nc.tensor.transpose(ps_t[:], pch[:], ident_f[:])
        nc.vector.tensor_copy(projT[:, c * P:(c + 1) * P], ps_t[:])
    nc.vector.tensor_scalar_mul(projT[:], projT[:], sc_b[:, 0:1])
    projT_b = const.tile([P, M], BF16)
    nc.vector.tensor_copy(projT_b[:], projT[:])
    ps_setup.release()

    ps_small = ctx.enter_context(tc.tile_pool(name="ps_small", bufs=2, space="PSUM"))
    ps_big = ctx.enter_context(tc.tile_pool(name="ps_big", bufs=3, space="PSUM"))

    # main loop, processed in PAIRS of 128-token blocks to halve the
    # per-stage semaphore handoffs (which otherwise latency-bind the PE
    # phase): two transposes share a PSUM tile and one eT copy; two
    # matmuls share a 2-bank PSUM tile, one bf16 copy and one 256KB
    # contiguous DMA.  Copies alternate DVE/ACT.  Block b holds tokens
    # 128b..128b+127 in order, so all output DMAs are sequential.
    NPAIR = NB // 2
    ets = {}

    def emit_trans(pb):
        ps_et = ps_small.tile([P, 2, P], F32, space="PSUM",
                              tag="ps_et", name=f"ps_et{pb}")
        nc.tensor.transpose(ps_et[:, 0, :], g_sb[:, 2 * pb, :], ident_f[:])
        nc.tensor.transpose(ps_et[:, 1, :], g_sb[:, 2 * pb + 1, :], ident_f[:])
        et = et_pool.tile([P, 2, P], BF16, tag="et", name=f"et{pb}")
        if pb % 2:
            nc.scalar.copy(et[:], ps_et[:])
        else:
            nc.vector.tensor_copy(et[:], ps_et[:])
        ets[pb] = et

    def emit_mm(pb):
        et = ets.pop(pb)
        ps_o = ps_big.tile([P, 2, M], F32, space="PSUM", tag="ps_o",
                           name=f"ps_o{pb}")
        nc.tensor.matmul(ps_o[:, 0, :], lhsT=et[:, 0, :], rhs=projT_b[:],
                         start=True, stop=True)
        nc.tensor.matmul(ps_o[:, 1, :], lhsT=et[:, 1, :], rhs=projT_b[:],
                         start=True, stop=True)
        o2 = o_pool.tile([P, 2, M], BF16, tag="o_sb", name=f"o2_{pb}")
        # split the pair's PSUM->SBUF copy across both engines (halves the
        # copy latency on the critical path; keeps ACT/DVE evenly loaded)
        nc.scalar.copy(o2[:, 0, :], ps_o[:, 0, :])
        nc.vector.tensor_copy(o2[:, 1, :], ps_o[:, 1, :])
        dst = out_ap[2 * P * pb:2 * P * (pb + 1), :]
        dst = dst.rearrange("(g p) m -> p g m", g=2)
        nc.sync.dma_start(dst, o2[:])

    for pb in range(NPAIR):
        emit_trans(pb)
        if pb >= LAG:
            emit_mm(pb - LAG)
    for pb in range(NPAIR - LAG, NPAIR):
        emit_mm(pb)


_CACHE: dict = {}
DEBUG = False    # dump idx/cur/prv tiles to DRAM for stage checking


def _build(key: int = 0):
    if key in _CACHE:
        return _CACHE[key]
    nc = bacc.Bacc("TRN2", target_bir_lowering=False, debug=False,
                   num_swdge_queues=N_QUEUES, dynamic_dma_scratch_size=131072)
    tok = nc.dram_tensor("token_ids", [2, P, SPT], I32, kind="ExternalInput").ap()
    table = nc.dram_tensor("embed_weight", [V, D], F32, kind="ExternalInput").ap()
    proj = nc.dram_tensor("proj_weight", [M, D], F32, kind="ExternalInput").ap()
    scale = nc.dram_tensor("scale", [1, 1], F32, kind="ExternalInput").ap()
    out = nc.dram_tensor("out", [S, M], BF16, kind="ExternalOutput").ap()
    dbg = None
    if DEBUG:
        dbg = {
            "idx": nc.dram_tensor("idx_dbg", [P, SPT], I16,
                                  kind="ExternalOutput").ap(),
            "cur": nc.dram_tensor("cur_dbg", [P, SPT], I32,
                                  kind="ExternalOutput").ap(),
            "prv": nc.dram_tensor("prv_dbg", [P, SPT], I32,
                                  kind="ExternalOutput").ap(),
        }
    with tile.TileContext(nc) as tc:
        with ExitStack() as ctx:
            body(ctx, tc, out, tok, table, proj, scale, dbg=dbg)
    nc.compile()
    _CACHE[key] = nc
    return nc


def stage_tokens(row: np.ndarray) -> np.ndarray:
    """[S] int token row -> [2, 128, SPT] int32 16-wrap (cur, prev) tiles,
    pre-replicated x8 across the partition dim (the gather requires its idx
    rows replicated per GpSimd core pair, and the hash then uses all 128
    DVE lanes)."""
    t32 = row.astype(np.int32)          # values < 2^31; lo-word == value
    prev = np.empty_like(t32)
    prev[0] = 0
    prev[1:] = t32[:-1]
    cur_w = np.tile(t32.reshape(SPT, 16).T, (8, 1))
    prv_w = np.tile(prev.reshape(SPT, 16).T, (8, 1))
    return np.ascontiguousarray(np.stack([cur_w, prv_w]))


def kernel(token_ids: np.ndarray, embed_weight: np.ndarray,
           proj_weight: np.ndarray, scale: np.ndarray) -> np.ndarray:
    token_ids = np.ascontiguousarray(token_ids)
    assert token_ids.shape == (B, S), token_ids.shape
    table = np.ascontiguousarray(embed_weight, dtype=np.float32)
    proj = np.ascontiguousarray(proj_weight, dtype=np.float32)
    sc = np.asarray(scale, dtype=np.float32).reshape(1, 1)

    nc = _build()
    in_maps = [
        {
            "token_ids": stage_tokens(token_ids[i]),
            "embed_weight": table,
            "proj_weight": proj,
            "scale": sc,
        }
        for i in range(B)
    ]
    res = run_bass_kernel_spmd(nc, in_maps, core_ids=list(range(B)))
    return np.stack([np.asarray(r["out"]).astype(np.float32)
                     for r in res.results], axis=0)


# revision 32
# speedup vs baseline: 1.4429x; 1.0126x over previous
"""Trainium2 Bass kernel: BigramHashEmbedding (hash -> embed gather -> proj -> scale).

Computation (per batch row, one NeuronCore per row, 8 rows total):
    h[0]  = 10239
    h[j]  = (36313*t[j] ^ 27191*t[j-1]) % 10239          (int32, j >= 1)
    e     = embed_weight[h]                               [S, 128] gather
    out   = (e @ proj_weight.T) * scale                   [S, 512]

Device strategy per core (S = 8192 tokens):
  * dma_gather unwraps its index tile column-major over 16 partitions
    (slot k <- idx[k%16, k//16]), so the host stages tokens in a 16-wrap
    layout (tok16[p, s] = t[16s + p], plus a one-shifted copy for the
    bigram's previous token; both are pure permutations of the int32 index
    tensor, staged as one [2, 16, 512] input).  With this layout gather
    slot k maps to token k exactly: gathered rows land as
    g_sb[p, b, :] = e[token 128b + p], the PE transpose of block b yields
    eT in plain token order, and every output DMA writes 128 contiguous
    rows (fully sequential HBM addresses).
  * the device loads the two wrapped tiles with contiguous 2KB-per-
    partition runs and broadcasts them x8 across the 128 partitions (the
    gather needs its idx rows replicated per GpSimd core pair; the hash
    then runs on all 128 DVE lanes).
  * the bigram hash runs on DVE/ACT with fp32-exact arithmetic: products
    are split (36313 = 141*256 + 217, 27191 = 106*256 + 55) so every
    arithmetic op stays below 2^24 (the vector ALU is fp32 internally);
    >=2^24 values only pass through bitwise ops, which are bit-exact.
    mod-10239 is a limb decomposition X = u*2^21 + v*2^8 + w ->
    y = u*8396 + (v<<8) + w (y < 2^24) plus one fp32 reciprocal-multiply
    quotient; the HW float->int converter rounds to nearest, so a single
    +m fixup suffices.
  * the embed table is converted once to bf16 in DRAM (cast-during-DMA on
    SWDGE, split into 4 queue-parallel chunks to shorten the startup
    serialization).  Eight dma_gathers (1024 rows each) fetch rows into
    [128, 64, 128] bf16.  (The transpose=True gather mode would skip the
    PE transposes below, but it routes through the shared XBAR: its
    descriptor generation costs ~8.5 ns/row and concurrent transposed
    gathers on different queues corrupt each other, so serialized it is
    ~70 us for 8K rows -- measured.  Plain gathers + PE transposes win.)
  * per 128-token block: bf16 PE transpose (identity) -> PSUM -> bf16 eT
    in SBUF (DVE/ACT alternating copy), then PE matmul eT.T @ projT_bf16
    -> PSUM f32 -> bf16 copy into a 2-block SBUF group (DVE/ACT
    alternating) -> one HWDGE DMA per 2 blocks (256KB contiguous).  The
    output tensor is bf16; the host upcasts to f32 (tolerance ~2e-2, bf16
    out adds ~2e-3).  Transposes run LAG blocks ahead of the matmuls so
    the eT copy stays off the PE's in-order critical path.
  * proj [512, 128] is transposed on the PE at setup into projT [128,
    512], pre-scaled by `scale` (broadcast via a K=1 matmul), cast bf16.

SWDGE semaphore lanes are round-robin (8) and lock to one queue each, so
every SWDGE DMA uses queue = emission_index % N_QUEUES to keep lane->queue
stable across the wrap (12 SWDGE DMAs: 4 conversion chunks + 8 gathers).
"""

from contextlib import ExitStack

import numpy as np

import concourse.bacc as bacc
import concourse.bass as bass
import concourse.mybir as mybir
import concourse.tile as tile
from concourse.bass_utils import run_bass_kernel_spmd
from concourse.masks import make_identity

AL = mybir.AluOpType
F32 = mybir.dt.float32
BF16 = mybir.dt.bfloat16
I32 = mybir.dt.int32
I16 = mybir.dt.int16

B = 8           # batch rows == cores
S = 8192        # tokens per core
V = 10240       # hash table rows
D = 128         # embed dim
M = 512         # model dim
P = 128
MOD = 10239     # hash modulus (HASH_SIZE - 1)
SPT = S // 16   # 16-wrap columns = 512
NG = 8          # gathers
IPG = S // NG   # idxs per gather = 1024
CPG = IPG // 16  # idx columns per gather = 64
NB = S // P     # 128-token blocks = 64
BPG = IPG // P  # matmul blocks per gather = 8
HASH_CHUNKS = (256, 256)   # two wide chunks: ACT muls of c2 overlap DVE c1
assert sum(HASH_CHUNKS) == SPT

# 36313 = 141*256 + 217 ; 27191 = 106*256 + 55
A_HI, A_LO = 141, 217
B_HI, B_LO = 106, 55
C21 = 8396      # 2^21 mod 10239
INV_M = 1.0 / MOD

USE_ACT_MUL = True   # run the big hash multiplies on the Scalar (ACT) engine
N_QUEUES = 4         # SWDGE queues (ucode MAX_SWDGE_QUEUES=4)
SIM_COMPAT = False   # add the >=MOD fixup (only needed under CoreSim's trunc convert)
LAG = 6              # transpose runs LAG pairs ahead of the matmul


def _mul(nc, out, in_, const):
    if USE_ACT_MUL:
        nc.scalar.mul(out, in_, float(const))
    else:
        nc.vector.tensor_scalar_mul(out, in_, float(const))


def _hash_chunk(nc, tmp, idx, cur, prv, mask, offs, cs, n):
    """Emit ops computing idx[:, cs:cs+n] (int16 hash values).

    cur: [128, SPT] int32, cur[p, s] = t[16s + p%16]   (x8 replicas)
    prv: [128, SPT] int32, prv[p, s] = t[16s + p%16 - 1] (0 at (p%16==0, 0))
    mask: [128, 1] int32, (p % 16) != 0.
    offs: [128, 1] int32, 10239 * (p % 16 == 0).
    """
    tcur = cur[:, cs:cs + n]
    tprev = prv[:, cs:cs + n]
    p1 = tmp.tile([P, n], I32, tag=f"p1_{n}")
    p2 = tmp.tile([P, n], I32, tag=f"p2_{n}")
    q1 = tmp.tile([P, n], I32, tag=f"q1_{n}")
    q2 = tmp.tile([P, n], I32, tag=f"q2_{n}")
    _mul(nc, p1[:], tcur, A_LO)
    _mul(nc, p2[:], tcur, A_HI)
    _mul(nc, q1[:], tprev, B_LO)
    _mul(nc, q2[:], tprev, B_HI)

    # A>>8 = p2 + (p1>>8);  B>>8 = q2 + (q1>>8)   (both < 2^23, exact)
    ah = tmp.tile([P, n], I32, tag=f"ah_{n}")
    bh = tmp.tile([P, n], I32, tag=f"bh_{n}")
    t1 = tmp.tile([P, n], I32, tag=f"t1_{n}")
    nc.vector.tensor_single_scalar(t1[:], p1[:], 8, op=AL.logical_shift_right)
    nc.vector.tensor_add(ah[:], t1[:], p2[:])
    nc.vector.tensor_single_scalar(t1[:], q1[:], 8, op=AL.logical_shift_right)
    nc.vector.tensor_add(bh[:], t1[:], q2[:])
    # X>>8 and X low byte (in low 8 bits of xl)
    xh = tmp.tile([P, n], I32, tag=f"xh_{n}")
    xl = tmp.tile([P, n], I32, tag=f"xl_{n}")
    nc.vector.tensor_tensor(xh[:], ah[:], bh[:], op=AL.bitwise_xor)
    nc.vector.tensor_tensor(xl[:], p1[:], q1[:], op=AL.bitwise_xor)

    # y = (xh>>13)*8396 + ((xh & 8191) << 8) + (xl & 255)   ( < 2^24 )
    w1 = tmp.tile([P, n], I32, tag=f"w1_{n}")
    w2 = tmp.tile([P, n], I32, tag=f"w2_{n}")
    nc.vector.tensor_single_scalar(w1[:], xh[:], 13, op=AL.logical_shift_right)
    nc.vector.tensor_scalar_mul(w1[:], w1[:], float(C21))
    nc.vector.tensor_scalar(w2[:], xh[:], 8191, 8,
                            op0=AL.bitwise_and, op1=AL.logical_shift_left)
    w3 = tmp.tile([P, n], I32, tag=f"w3_{n}")
    nc.vector.tensor_add(w3[:], w1[:], w2[:])
    y = tmp.tile([P, n], I32, tag=f"y_{n}")
    nc.vector.tensor_single_scalar(y[:], xl[:], 255, op=AL.bitwise_and)
    nc.vector.tensor_add(y[:], y[:], w3[:])

    # r = y - rne(y/m)*m  (HW converter is round-to-nearest => r < m always)
    qt = tmp.tile([P, n], I32, tag=f"qt_{n}")
    _mul(nc, qt[:], y[:], INV_M)
    r = tmp.tile([P, n], I32, tag=f"r_{n}")
    nc.vector.scalar_tensor_tensor(r[:], qt[:], -float(MOD), y[:],
                                   op0=AL.mult, op1=AL.add)
    if SIM_COMPAT:
        f1 = tmp.tile([P, n], I32, tag=f"f1_{n}")
        nc.vector.tensor_single_scalar(f1[:], r[:], float(MOD), op=AL.is_ge)
        nc.vector.scalar_tensor_tensor(r[:], f1[:], -float(MOD), r[:],
                                       op0=AL.mult, op1=AL.add)
    f2 = tmp.tile([P, n], I32, tag=f"f2_{n}")
    nc.vector.tensor_single_scalar(f2[:], r[:], 0.0, op=AL.is_lt)
    # final fixup writes straight into the int16 idx tile (cast on store)
    nc.vector.scalar_tensor_tensor(idx[:, cs:cs + n], f2[:], float(MOD),
                                   r[:], op0=AL.mult, op1=AL.add)

    if cs == 0:
        # token 0 (partition p%16==0, col 0): h = MOD
        nc.vector.tensor_mul(idx[:, 0:1], idx[:, 0:1], mask[:])
        nc.vector.tensor_add(idx[:, 0:1], idx[:, 0:1], offs[:])


def body(ctx: ExitStack, tc: tile.TileContext, out_ap, tok_ap, table_ap,
         proj_ap, scale_ap, dbg=None):
    """Emit the per-core kernel.  tok_ap is int32 [2, 16, SPT]: the host-
    staged 16-wrap current-token and previous-token tiles."""
    nc = tc.nc

    const = ctx.enter_context(tc.tile_pool(name="const", bufs=1))
    tmp = ctx.enter_context(tc.tile_pool(name="tmp", bufs=1))
    gpool = ctx.enter_context(tc.tile_pool(name="gpool", bufs=1))
    et_pool = ctx.enter_context(tc.tile_pool(name="et", bufs=8))
    o_pool = ctx.enter_context(tc.tile_pool(name="osb", bufs=3))

    # ---- setup FIRST: projT (transposed, pre-scaled, bf16).  Emitted
    # before the hash so its DVE footprint (one mul) clears the in-order
    # DVE queue before hash ops land: interleaved late, its PSUM copies
    # stall the DVE mid-hash for ~7.5us waiting on PE transposes, and the
    # gathers' DVE-semaphore waits inherit the stall (measured).  The
    # `scale` factor is folded into the transpose identity (sc * I), so
    # the proj chunks come out of the PE already scaled; ACT does the
    # PSUM->bf16 copies. ----
    ps_setup = tc.alloc_tile_pool(name="ps_setup", bufs=1, space="PSUM")
    ident_f = const.tile([P, P], F32)
    make_identity(nc, ident_f[:])

    # scale broadcast [1,1] -> [128,1] via K=1 matmul with a ones row
    sc_in = const.tile([1, 1], F32)
    nc.sync.dma_start(sc_in[:], scale_ap)
    ones = const.tile([1, P], F32)
    nc.gpsimd.memset(ones[:], 1.0)
    ps_sc = ps_setup.tile([P, 1], F32, space="PSUM", tag="ps_sc")
    nc.tensor.matmul(ps_sc[:], lhsT=ones[:], rhs=sc_in[:], start=True, stop=True)
    sc_b = const.tile([P, 1], F32)
    nc.vector.tensor_copy(sc_b[:], ps_sc[:])
    ident_sc = const.tile([P, P], F32)
    nc.vector.tensor_scalar_mul(ident_sc[:], ident_f[:], sc_b[:, 0:1])

    projT_b = const.tile([P, M], BF16)
    for c in range(M // P):
        pch = tmp.tile([P, P], F32, tag="pch")
        nc.sync.dma_start(pch[:], proj_ap[c * P:(c + 1) * P, :])
        ps_t = ps_setup.tile([P, P], F32, space="PSUM", tag="ps_t")
        # regular matmul (not transpose mode): pch.T @ (sc*I) = sc*projT
        nc.tensor.matmul(ps_t[:], lhsT=pch[:], rhs=ident_sc[:],
                         start=True, stop=True)
        nc.scalar.copy(projT_b[:, c * P:(c + 1) * P], ps_t[:])
    ps_setup.release()

    # ---- tokens: host-staged 16-wrap tiles, pre-replicated x8 so a
    # single contiguous DMA per tile makes them hash-ready ----
    cur = const.tile([P, SPT], I32)
    prv = const.tile([P, SPT], I32)
    nc.sync.dma_start(cur[:], tok_ap[0])
    nc.scalar.dma_start(prv[:], tok_ap[1])

    # partition masks for the token-0 override
    pi = const.tile([P, 1], I32)
    nc.gpsimd.iota(pi[:], pattern=[[0, 1]], base=0, channel_multiplier=1)
    m32 = const.tile([P, 1], I32)
    nc.vector.tensor_single_scalar(m32[:], pi[:], 15, op=AL.bitwise_and)
    nc.vector.tensor_single_scalar(m32[:], m32[:], 0.0, op=AL.not_equal)
    mask = const.tile([P, 1], I16)
    nc.vector.tensor_copy(mask[:], m32[:])
    offs = const.tile([P, 1], I16)
    nc.vector.tensor_scalar(offs[:], m32[:], -float(MOD), float(MOD),
                            op0=AL.mult, op1=AL.add)

    idx = const.tile([P, SPT], I16)
    # g_sb[p, b, :] = embed_f32[h(token 128b + p), :] -- gathered straight
    # from the fp32 table (512B rows cost the same DMA descriptor time as
    # 256B ones, and skipping the bf16 pre-conversion removes a 7.9 MB
    # DMA flood that otherwise stalls startup by ~25 us).  The bf16 cast
    # happens for free in the eT PSUM->SBUF copy.
    g_sb = gpool.tile([P, NB, P], F32)

    # hash chunks first, all gathers after: interleaving them makes the
    # in-order Pool stream park inside long DMAGatherAnt slices while DVE
    # hash ops stall against the Q7 descriptor-gen (measured ~7us bubbles)
    cs = 0
    for n in HASH_CHUNKS:
        _hash_chunk(nc, tmp, idx, cur, prv, mask, offs, cs, n)
        cs += n
    for g in range(NG):
        nc.gpsimd.dma_gather(
            g_sb[:, BPG * g:BPG * (g + 1), :],
            table_ap,
            idx[:, CPG * g:CPG * (g + 1)],
            num_idxs=IPG,
            num_idxs_reg=IPG,
            elem_size=D,
            single_packet=False,
            queue_num=g % N_QUEUES,
        )

    if dbg is not None:
        nc.sync.dma_start(dbg["idx"], idx[:])
        nc.sync.dma_start(dbg["cur"], cur[:])
        nc.sync.dma_start(dbg["prv"], prv[:])

    ps_small = ctx.enter_context(tc.tile_pool(name="ps_small", bufs=2, space="PSUM"))
    ps_big = ctx.enter_context(tc.tile_pool(name="ps_big", bufs=3, space="PSUM"))

    # main loop, processed in PAIRS of 128-token blocks to halve the
    # per-stage semaphore handoffs (which otherwise latency-bind the PE
    # phase): two transposes share a PSUM tile and one eT copy; two
    # matmuls share a 2-bank PSUM tile, one bf16 copy and one 256KB
    # contiguous DMA.  Copies alternate DVE/ACT.  Block b holds tokens
    # 128b..128b+127 in order, so all output DMAs are sequential.
    NPAIR = NB // 2
    ets = {}

    def emit_trans(pb):
        ps_et = ps_small.tile([P, 2, P], F32, space="PSUM",
                              tag="ps_et", name=f"ps_et{pb}")
        nc.tensor.transpose(ps_et[:, 0, :], g_sb[:, 2 * pb, :], ident_f[:])
        nc.tensor.transpose(ps_et[:, 1, :], g_sb[:, 2 * pb + 1, :], ident_f[:])
        et = et_pool.tile([P, 2, P], BF16, tag="et", name=f"et{pb}")
        if pb % 2:
            nc.scalar.copy(et[:], ps_et[:])
        else:
            nc.vector.tensor_copy(et[:], ps_et[:])
        ets[pb] = et

    def emit_mm(pb):
        et = ets.pop(pb)
        ps_o = ps_big.tile([P, 2, M], F32, space="PSUM", tag="ps_o",
                           name=f"ps_o{pb}")
        nc.tensor.matmul(ps_o[:, 0, :], lhsT=et[:, 0, :], rhs=projT_b[:],
                         start=True, stop=True)
        nc.tensor.matmul(ps_o[:, 1, :], lhsT=et[:, 1, :], rhs=projT_b[:],
                         start=True, stop=True)
        o2 = o_pool.tile([P, 2, M], BF16, tag="o_sb", name=f"o2_{pb}")
        # split the pair's PSUM->SBUF copy across both engines (halves the
        # copy latency on the critical path; keeps ACT/DVE evenly loaded)
        nc.scalar.copy(o2[:, 0, :], ps_o[:, 0, :])
        nc.vector.tensor_copy(o2[:, 1, :], ps_o[:, 1, :])
        dst = out_ap[2 * P * pb:2 * P * (pb + 1), :]
        dst = dst.rearrange("(g p) m -> p g m", g=2)
        nc.sync.dma_start(dst, o2[:])

    for pb in range(NPAIR):
        emit_trans(pb)
        if pb >= LAG:
            emit_mm(pb - LAG)
    for pb in range(NPAIR - LAG, NPAIR):
        emit_mm(pb)


_CACHE: dict = {}
DEBUG = False    # dump idx/cur/prv tiles to DRAM for stage checking


def _build(key: int = 0):
    if key in _CACHE:
        return _CACHE[key]
    nc = bacc.Bacc("TRN2", target_bir_lowering=False, debug=False,
                   num_swdge_queues=N_QUEUES, dynamic_dma_scratch_size=131072)
    tok = nc.dram_tensor("token_ids", [2, P, SPT], I32, kind="ExternalInput").ap()
    table = nc.dram_tensor("embed_weight", [V, D], F32, kind="ExternalInput").ap()
    proj = nc.dram_tensor("proj_weight", [M, D], F32, kind="ExternalInput").ap()
    scale = nc.dram_tensor("scale", [1, 1], F32, kind="ExternalInput").ap()
    out = nc.dram_tensor("out", [S, M], BF16, kind="ExternalOutput").ap()
    dbg = None
    if DEBUG:
        dbg = {
            "idx": nc.dram_tensor("idx_dbg", [P, SPT], I16,
                                  kind="ExternalOutput").ap(),
            "cur": nc.dram_tensor("cur_dbg", [P, SPT], I32,
                                  kind="ExternalOutput").ap(),
            "prv": nc.dram_tensor("prv_dbg", [P, SPT], I32,
                                  kind="ExternalOutput").ap(),
        }
    with tile.TileContext(nc) as tc:
        with ExitStack() as ctx:
            body(ctx, tc, out, tok, table, proj, scale, dbg=dbg)
    nc.compile()
    _CACHE[key] = nc
    return nc


def stage_tokens(row: np.ndarray) -> np.ndarray:
    """[S] int token row -> [2, 128, SPT] int32 16-wrap (cur, prev) tiles,
    pre-replicated x8 across the partition dim (the gather requires its idx
    rows replicated per GpSimd core pair, and the hash then uses all 128
    DVE lanes)."""
    t32 = row.astype(np.int32)          # values < 2^31; lo-word == value
    prev = np.empty_like(t32)
    prev[0] = 0
    prev[1:] = t32[:-1]
    cur_w = np.tile(t32.reshape(SPT, 16).T, (8, 1))
    prv_w = np.tile(prev.reshape(SPT, 16).T, (8, 1))
    return np.ascontiguousarray(np.stack([cur_w, prv_w]))


def kernel(token_ids: np.ndarray, embed_weight: np.ndarray,
           proj_weight: np.ndarray, scale: np.ndarray) -> np.ndarray:
    token_ids = np.ascontiguousarray(token_ids)
    assert token_ids.shape == (B, S), token_ids.shape
    table = np.ascontiguousarray(embed_weight, dtype=np.float32)
    proj = np.ascontiguousarray(proj_weight, dtype=np.float32)
    sc = np.asarray(scale, dtype=np.float32).reshape(1, 1)

    nc = _build()
    in_maps = [
        {
            "token_ids": stage_tokens(token_ids[i]),
            "embed_weight": table,
            "proj_weight": proj,
            "scale": sc,
        }
        for i in range(B)
    ]
    res = run_bass_kernel_spmd(nc, in_maps, core_ids=list(range(B)))
    return np.stack([np.asarray(r["out"]).astype(np.float32)
                     for r in res.results], axis=0)
